# revision 11
# baseline (speedup 1.0000x reference)
"""Trainium2 Bass kernel for nn_CustomTransformerDecoderLayer_3753801416834.

Strategy (pure data-parallel over batch, 4 batches per core on 8 cores):
  * grid_sample commutes with the 1x1 value conv (both linear) -> gather only
    the 2x2 bilinear corner patches from bev_feature and project afterwards.
    Host passes bev in NHWC layout so one gather descriptor is 512 contiguous
    floats (2 x-adjacent pixels x 256 channels) -> 1280 descriptors/core.
  * ego attention has Lk=1 so softmax == 1: collapses to a broadcast linear.
  * point-attention weights fold into the bilinear corner weights (one 32-way
    weighted reduce per query), and bias-terms fold into K=1 matmuls.
  * agent attention is batched across 4 batches x 8 heads as block-diagonal
    [80 x 128] score matrices with a multiplicative 0/1 mask.
Host-side work is limited to sharding, layout permutation and integer corner
index/weight prep; all tensor math runs on-device.
"""

import sys

import numpy as np

for _p in ("/opt/trn_rl_repo",):
    if _p not in sys.path:
        sys.path.insert(0, _p)

import concourse.bass as bass  # noqa: E402
import concourse.mybir as mybir  # noqa: E402
import concourse.tile as tile  # noqa: E402
from concourse import bacc, bass_utils  # noqa: E402
from concourse.masks import make_identity  # noqa: E402

F32 = mybir.dt.float32
I32 = mybir.dt.int32
AX = mybir.AxisListType.X
OP = mybir.AluOpType
AF = mybir.ActivationFunctionType

NCORES = 8
B, M, P, D, A, FF = 32, 20, 8, 256, 32, 1024
BL = B // NCORES          # local batches per core = 4
R = BL * M                # token rows per core = 80
AT = BL * A               # agent tokens per core = 128
NH, DH = 8, 32            # heads, head dim
HW = 128                  # bev spatial size
BEV_RANGE = 32.0
SCALE = 1.0 / float(np.sqrt(DH))

# ---- bias table layout (host & device share) -------------------------------
BIAS_SPEC = [
    ("aq_bq", 256), ("aq_bk", 256), ("aq_bv", 256), ("aq_bo", 256),
    ("eq_bv", 256), ("eq_bo", 256),
    ("ffn_b1", 1024), ("ffn_b2", 256),
    ("mod_b", 512),
    ("cls_b1", 256), ("cls_b2", 256), ("cls_b3", 1),
    ("reg_b1", 256), ("reg_b2", 256), ("reg_b3", 24),
    ("bev_attn_b", 8), ("bev_out_b", 256),
]
BOFF = {}
_o = 0
for _n, _s in BIAS_SPEC:
    BOFF[_n] = _o
    _o += _s
NB = _o

LN_SPEC = ["n1_g", "n1_b", "n2_g", "n2_b", "n3_g", "n3_b",
           "cls_ln1_g", "cls_ln1_b", "cls_ln2_g", "cls_ln2_b"]
LOFF = {n: i * 256 for i, n in enumerate(LN_SPEC)}
NL = 256 * len(LN_SPEC)

WEIGHTS_2D = [  # name -> (K, N); loaded as K/128 chunks of [128, N]
    ("aq_wq", 256, 256), ("aq_wk", 256, 256), ("aq_wv", 256, 256),
    ("aq_wo", 256, 256), ("eq_wv", 256, 256), ("eq_wo", 256, 256),
    ("ffn_w1", 256, 1024), ("ffn_w2", 1024, 256), ("mod_w", 256, 512),
    ("cls_w1", 256, 256), ("cls_w2", 256, 256), ("cls_w3", 256, 1),
    ("reg_w1", 256, 256), ("reg_w2", 256, 256), ("reg_w3", 256, 24),
    ("bev_value_w", 256, 256), ("bev_out_w", 256, 256), ("bev_attn_w", 256, 8),
]


def build_program(enable_asserts: bool = False):
    nc = bacc.Bacc(
        "TRN2",
        target_bir_lowering=False,
        debug=False,
        enable_asserts=enable_asserts,
        num_devices=NCORES,
    )

    def din(name, shape, dtype=F32):
        return nc.dram_tensor(name, list(shape), dtype, kind="ExternalInput")

    bev_h = din("bev", (BL * HW * HW, D))
    traj_h = din("traj", (R, D))
    agents_h = din("agents", (AT, D))
    ego_h = din("ego", (BL, D))
    time_h = din("time_emb", (BL, D))
    noisy_h = din("noisy", (R, P * 2))
    offs_h = din("offs", (R, P * 2), I32)
    w4_h = din("w4", (R, P * 4))
    amask_h = din("amask", (R, NH * AT))
    bsel_h = din("bsel", (BL, R))
    bigb_h = din("bigb", (1, NB))
    lnv_h = din("lnv", (1, NL))
    bvp_h = din("bv_pack", (128, 2))
    w_h = {n: din(n, (k, nn)) for n, k, nn in WEIGHTS_2D}
    out_h = nc.dram_tensor("out", [R, 25], F32, kind="ExternalOutput")

    with tile.TileContext(nc) as tc:
        with (
            tc.tile_pool(name="w", bufs=1) as wp,
            tc.tile_pool(name="act", bufs=1) as ap_,
            tc.tile_pool(name="tmp", bufs=4) as tp,
            tc.tile_pool(name="tt", bufs=1) as ttp,
            tc.tile_pool(name="ps", bufs=8, space="PSUM") as pp,
        ):
            # ---------------- constants / weights ----------------
            ident = wp.tile([128, 128], F32, tag="ident")
            make_identity(nc, ident[:])
            ones = wp.tile([1, 128], F32, tag="ones")
            nc.gpsimd.memset(ones[:], 1.0)
            epsc = wp.tile([128, 1], F32, tag="epsc")
            nc.gpsimd.memset(epsc[:], 1e-5)

            W = {}
            for wi, (name, K, N) in enumerate(WEIGHTS_2D):
                nchunk = K // 128
                t = wp.tile([128, nchunk * N], F32, tag=f"w_{name}", name=f"w_{name}")
                eng = nc.sync if wi % 2 == 0 else nc.scalar
                eng.dma_start(
                    out=t[:].rearrange("p (c n) -> p c n", c=nchunk),
                    in_=w_h[name].ap().rearrange("(c p) n -> p c n", p=128))
                W[name] = [t[:, i * N:(i + 1) * N] for i in range(nchunk)]

            bigb = wp.tile([1, NB], F32, tag="bigb")
            nc.sync.dma_start(out=bigb[:], in_=bigb_h.ap())
            lnv = wp.tile([1, NL], F32, tag="lnv")
            nc.sync.dma_start(out=lnv[:], in_=lnv_h.ap())
            bvp = wp.tile([128, 2], F32, tag="bvp")
            nc.sync.dma_start(out=bvp[:], in_=bvp_h.ap())

            def bslice(name, lo=0, size=None):
                off = BOFF[name] + lo
                if size is None:
                    size = dict((n, s) for n, s in BIAS_SPEC)[name]
                return bigb[0:1, off:off + size]

            # replicate LN vectors across partitions via K=1 matmuls
            lnr = wp.tile([128, NL], F32, tag="lnr")
            for s in range(NL // 512):
                ps = pp.tile([128, 512], F32, tag="ps")
                nc.tensor.matmul(out=ps[:], lhsT=ones[0:1, :128],
                                 rhs=lnv[0:1, s * 512:(s + 1) * 512],
                                 start=True, stop=True)
                nc.vector.tensor_copy(out=lnr[:, s * 512:(s + 1) * 512], in_=ps[:])

            def lslice(name):
                return lnr[:R, LOFF[name]:LOFF[name] + 256]

            # ---------------- inputs ----------------
            _ldc = [0]
            def load(h, shape, dtype=F32, tag=None):
                t = ap_.tile(list(shape), dtype, tag=tag or h.name, name=tag or h.name)
                eng = nc.sync if _ldc[0] % 2 == 0 else nc.scalar
                _ldc[0] += 1
                eng.dma_start(out=t[:], in_=h.ap())
                return t

            traj = load(traj_h, (R, D))
            agents = load(agents_h, (AT, D))
            ego = load(ego_h, (BL, D))
            time_e = load(time_h, (BL, D))
            noisy = load(noisy_h, (R, P * 2))
            offs = load(offs_h, (R, P * 2), I32)
            w4 = load(w4_h, (R, P * 4))
            amask = load(amask_h, (R, NH * AT))
            bsel = load(bsel_h, (BL, R))

            # gathered 2x2xC corner patches: per (query,point,ycorner) 512 floats
            # HW indirect DMA consumes ONE index per partition, gathering the
            # out-row free size contiguously -> one call per (point, ycorner).
            G = ap_.tile([R, P * 2 * 512], F32, tag="G")
            for k in range(P * 2):
                nc.gpsimd.indirect_dma_start(
                    out=G[:, k * 512:(k + 1) * 512], out_offset=None,
                    in_=bev_h.ap(),
                    in_offset=bass.IndirectOffsetOnAxis(ap=offs[:, k:k + 1], axis=0),
                )

            # ---------------- helpers ----------------
            def PS(p, f):
                return pp.tile([p, f], F32, tag="ps", name="ps")

            def sb(p, f, tag):
                return ap_.tile([p, f], F32, tag=tag, name=tag)

            def copy(dst_ap, src_ap, eng="v"):
                if eng == "v":
                    nc.vector.tensor_copy(out=dst_ap, in_=src_ap)
                else:
                    nc.scalar.copy(out=dst_ap, in_=src_ap)

            def transpose_to(src_ap, pdim, fdim, tag, eng="v"):
                """src [pdim, fdim] (sbuf) -> new sbuf tile [fdim, pdim]."""
                ps = PS(fdim, pdim)
                nc.tensor.transpose(out=ps[:], in_=src_ap, identity=ident[:pdim, :pdim])
                t = ttp.tile([fdim, pdim], F32, tag=tag, name=tag)
                copy(t[:], ps[:], eng)
                return t

            def transpose256(src, pdim, tagbase, eng="v"):
                return [transpose_to(src[:, i * 128:(i + 1) * 128], pdim, 128,
                                     f"{tagbase}{i}", eng) for i in range(2)]

            def tok_bias(ps_ap, name, n, start=False, stop=False, rows=R):
                """out[m, :] += bias (K=1 matmul, ones as lhsT)."""
                nc.tensor.matmul(out=ps_ap, lhsT=ones[0:1, :rows],
                                 rhs=bslice(name, 0, n), start=start, stop=stop)

            def layer_norm(src_ap, gname, bname, dst_ap, sfx):
                srow = sb(R, 1, f"ln_s{sfx}")
                nc.vector.reduce_sum(srow[:], src_ap, axis=AX)
                nm = sb(R, 1, f"ln_nm{sfx}")
                nc.scalar.activation(out=nm[:], in_=srow[:], func=AF.Copy,
                                     scale=-1.0 / D)
                xm = sb(R, D, f"ln_xm{sfx}")
                nc.vector.tensor_scalar_add(out=xm[:], in0=src_ap, scalar1=nm[:])
                sq = tp.tile([R, D], F32, tag="fmatmp", name="fmatmp")
                ssq = sb(R, 1, f"ln_q{sfx}")
                nc.scalar.activation(out=sq[:], in_=xm[:], func=AF.Square,
                                     accum_out=ssq[:])
                # rsqrt(var+eps) on DVE only: quake seed + 2 Newton steps
                v_ = sb(R, 1, f"ln_v{sfx}")
                nc.vector.tensor_scalar(out=v_[:], in0=ssq[:], scalar1=1.0 / D,
                                        scalar2=1e-5, op0=OP.mult, op1=OP.add)
                iv = sb(R, 1, f"ln_iv{sfx}")
                I32v = v_[:].bitcast(I32)
                nc.vector.tensor_scalar(out=iv[:].bitcast(I32), in0=I32v,
                                        scalar1=1, scalar2=None,
                                        op0=OP.arith_shift_right)
                rstd = sb(R, 1, f"ln_r{sfx}")
                nc.vector.tensor_scalar(out=rstd[:].bitcast(I32),
                                        in0=iv[:].bitcast(I32),
                                        scalar1=-1, scalar2=0x5f3759df,
                                        op0=OP.mult, op1=OP.add)
                for it_ in range(2):
                    yy = sb(R, 1, f"ln_y{sfx}{it_}")
                    nc.vector.tensor_tensor(out=yy[:], in0=rstd[:], in1=rstd[:],
                                            op=OP.mult)
                    nc.vector.tensor_tensor(out=yy[:], in0=yy[:], in1=v_[:],
                                            op=OP.mult)
                    nc.vector.tensor_scalar(out=yy[:], in0=yy[:], scalar1=-0.5,
                                            scalar2=1.5, op0=OP.mult, op1=OP.add)
                    nc.vector.tensor_tensor(out=rstd[:], in0=rstd[:], in1=yy[:],
                                            op=OP.mult)
                nc.vector.tensor_scalar_mul(out=xm[:], in0=xm[:], scalar1=rstd[:])
                nc.vector.tensor_tensor(out=dst_ap, in0=xm[:], in1=lslice(gname),
                                        op=OP.mult)
                nc.vector.tensor_tensor(out=dst_ap, in0=dst_ap, in1=lslice(bname),
                                        op=OP.add)

            # ---------------- BEV branch ----------------
            # point-attention weights ptw = softmax(traj @ bev_attn_w + b)
            trajT = transpose256(traj, R, "trajT")
            pw_ps = PS(R, P)
            nc.tensor.matmul(out=pw_ps[:], lhsT=trajT[0][:, :R], rhs=W["bev_attn_w"][0],
                             start=True, stop=False)
            nc.tensor.matmul(out=pw_ps[:], lhsT=trajT[1][:, :R], rhs=W["bev_attn_w"][1],
                             start=False, stop=False)
            tok_bias(pw_ps[:], "bev_attn_b", P, stop=True)
            nmax = sb(R, 1, "pw_nmax")
            nc.vector.reduce_max(nmax[:], pw_ps[:], axis=AX, negate=True)
            pexp = sb(R, P, "pw_exp")
            nc.scalar.activation(out=pexp[:], in_=pw_ps[:], func=AF.Exp,
                                 bias=nmax[:], scale=1.0)
            psum_ = sb(R, 1, "pw_sum")
            nc.vector.reduce_sum(psum_[:], pexp[:], axis=AX)
            prec = sb(R, 1, "pw_rec")
            nc.vector.reciprocal(prec[:], psum_[:])
            ptw = sb(R, P, "ptw")
            nc.vector.tensor_scalar_mul(out=ptw[:], in0=pexp[:], scalar1=prec[:])

            # combined corner weights cw[m, pt*4 + yc*2 + xc]
            cw = sb(R, P * 4, "cw")
            for p_ in range(P):
                nc.vector.tensor_scalar_mul(out=cw[:, 4 * p_:4 * p_ + 4],
                                            in0=w4[:, 4 * p_:4 * p_ + 4],
                                            scalar1=ptw[:, p_:p_ + 1])

            # gq[m, c] = sum_j cw[m, j] * G[m, j, c]   (4 parallel chains)
            accs = []
            for k in range(4):
                a = sb(R, D, f"gacc{k}")
                for ji, j in enumerate(range(k * 8, k * 8 + 8)):
                    pt, rem = divmod(j, 4)
                    yc, xc = divmod(rem, 2)
                    off = (pt * 2 + yc) * 512 + xc * 256
                    gsl = G[:, off:off + 256]
                    if ji == 0:
                        nc.scalar.activation(out=a[:], in_=gsl, func=AF.Copy,
                                             scale=cw[:, j:j + 1])
                    else:
                        t = tp.tile([R, D], F32, tag="fmatmp", name="fmatmp")
                        nc.scalar.activation(out=t[:], in_=gsl, func=AF.Copy,
                                             scale=cw[:, j:j + 1])
                        nc.vector.tensor_tensor(out=a[:], in0=a[:], in1=t[:], op=OP.add)
                accs.append(a)
            nc.vector.tensor_tensor(out=accs[0][:], in0=accs[0][:], in1=accs[1][:], op=OP.add)
            nc.vector.tensor_tensor(out=accs[2][:], in0=accs[2][:], in1=accs[3][:], op=OP.add)
            gq = sb(R, D, "gq")
            nc.vector.tensor_tensor(out=gq[:], in0=accs[0][:], in1=accs[2][:], op=OP.add)

            # s[m] = sum_j cw[m, j]  (validity-weighted bias scale), as [1, R]
            srow = sb(R, 1, "cw_s")
            nc.vector.reduce_sum(srow[:], cw[:], axis=AX)
            sT_ps = PS(1, R)
            nc.tensor.transpose(out=sT_ps[:], in_=srow[:, 0:1], identity=ident[:R, :R])
            sT = sb(1, R, "cw_sT")
            copy(sT[:], sT_ps[:], "s")

            # W2 = bev_value_w @ bev_out_w  (on device, one-time)
            WvT = []  # e-chunks [128e, 256c]
            for j in range(2):
                t = wp.tile([128, 256], F32, tag=f"WvT{j}", name=f"WvT{j}")
                for i in range(2):
                    ps = PS(128, 128)
                    nc.tensor.transpose(out=ps[:],
                                        in_=W["bev_value_w"][i][:, j * 128:(j + 1) * 128],
                                        identity=ident[:])
                    copy(t[:, i * 128:(i + 1) * 128], ps[:], "s")
                WvT.append(t)
            W2 = []
            for i in range(2):
                ps = PS(128, 256)
                for j in range(2):
                    nc.tensor.matmul(out=ps[:], lhsT=WvT[j][:, i * 128:(i + 1) * 128],
                                     rhs=W["bev_out_w"][j],
                                     start=(j == 0), stop=(j == 1))
                t = wp.tile([128, 256], F32, tag=f"W2_{i}", name=f"W2_{i}")
                copy(t[:], ps[:], "v")
                W2.append(t)

            # bvWo = bev_value_b @ bev_out_w  [1, 256]
            bw_ps = PS(1, 256)
            nc.tensor.matmul(out=bw_ps[:], lhsT=bvp[:, 0:1], rhs=W["bev_out_w"][0],
                             start=True, stop=False)
            nc.tensor.matmul(out=bw_ps[:], lhsT=bvp[:, 1:2], rhs=W["bev_out_w"][1],
                             start=False, stop=True)
            bvWo = sb(1, 256, "bvWo")
            copy(bvWo[:], bw_ps[:], "s")

            # traj1 = traj + gq @ W2 + s * bvWo + bev_out_b
            gqT = transpose256(gq, R, "gqT")
            t1_ps = PS(R, D)
            nc.tensor.matmul(out=t1_ps[:], lhsT=gqT[0][:, :R], rhs=W2[0][:], start=True, stop=False)
            nc.tensor.matmul(out=t1_ps[:], lhsT=gqT[1][:, :R], rhs=W2[1][:], start=False, stop=False)
            nc.tensor.matmul(out=t1_ps[:], lhsT=sT[0:1, :R], rhs=bvWo[0:1, :], start=False, stop=False)
            tok_bias(t1_ps[:], "bev_out_b", 256)
            nc.tensor.matmul(out=t1_ps[:], lhsT=ident[:R, :R], rhs=traj[:], start=False, stop=True)
            traj1 = sb(R, D, "traj1")
            copy(traj1[:], t1_ps[:], "v")

            # ---------------- agents cross-attention + LN1 ----------------
            agT = transpose256(agents, AT, "agT")
            tr1T = transpose256(traj1, R, "tr1T")

            def projT(wname, bname, rhs_tiles, ntok, tagbase):
                """x @ W + b, produced transposed: 4 chunks [64 dout, ntok]
                so per-head slices start at partition base 0 or 32."""
                outs = []
                for j in range(4):
                    ps = PS(64, ntok)
                    for i in range(2):
                        nc.tensor.matmul(out=ps[:],
                                         lhsT=W[wname][i][:, j * 64:(j + 1) * 64],
                                         rhs=rhs_tiles[i][:, :ntok],
                                         start=(i == 0), stop=False)
                    nc.tensor.matmul(out=ps[:], lhsT=bslice(bname, j * 64, 64),
                                     rhs=ones[0:1, :ntok], start=False, stop=True)
                    t = ttp.tile([64, ntok], F32, tag=f"{tagbase}{j}", name=f"{tagbase}{j}")
                    copy(t[:], ps[:], "s" if j % 2 else "v")
                    outs.append(t)
                return outs

            qT = projT("aq_wq", "aq_bq", tr1T, R, "qT")
            kT = projT("aq_wk", "aq_bk", agT, AT, "kT")

            # v_proj [AT, 256]
            v_ps = PS(AT, 256)
            nc.tensor.matmul(out=v_ps[:], lhsT=agT[0][:], rhs=W["aq_wv"][0], start=True, stop=False)
            nc.tensor.matmul(out=v_ps[:], lhsT=agT[1][:], rhs=W["aq_wv"][1], start=False, stop=False)
            tok_bias(v_ps[:], "aq_bv", 256, stop=True, rows=AT)
            vproj = sb(AT, 256, "vproj")
            copy(vproj[:], v_ps[:], "v")

            # scores -> masked softmax (no max-sub needed; values bounded)
            eall = sb(R, NH * AT, "eall")
            for h in range(NH):
                j, r0 = divmod(h, 2)
                r0 *= DH
                s_ps = PS(R, AT)
                nc.tensor.matmul(out=s_ps[:], lhsT=qT[j][r0:r0 + DH, :R],
                                 rhs=kT[j][r0:r0 + DH, :AT], start=True, stop=True)
                nc.scalar.activation(out=eall[:, h * AT:(h + 1) * AT], in_=s_ps[:],
                                     func=AF.Exp, scale=SCALE)
            nc.vector.tensor_tensor(out=eall[:], in0=eall[:], in1=amask[:], op=OP.mult)
            rs = sb(R, NH, "rs")
            nc.vector.reduce_sum(rs[:], eall[:].rearrange("p (h k) -> p h k", k=AT), axis=AX)
            rin = sb(R, NH, "rin")
            nc.vector.reciprocal(rin[:], rs[:])
            pn = sb(R, NH * AT, "pn")
            for h in range(NH):
                nc.vector.tensor_scalar_mul(out=pn[:, h * AT:(h + 1) * AT],
                                            in0=eall[:, h * AT:(h + 1) * AT],
                                            scalar1=rin[:, h:h + 1])

            attnT_sb = []
            for j in range(2):
                t = ttp.tile([128, R], F32, tag=f"attnT{j}", name=f"attnT{j}")
                for hh in range(4):
                    h = j * 4 + hh
                    pT_ps = PS(AT, R)
                    nc.tensor.transpose(out=pT_ps[:], in_=pn[:, h * AT:(h + 1) * AT],
                                        identity=ident[:R, :R])
                    pT = tp.tile([AT, R], F32, tag="pT", name="pT")
                    copy(pT[:], pT_ps[:], "s" if hh % 2 else "v")
                    a_ps = PS(DH, R)
                    nc.tensor.matmul(out=a_ps[:],
                                     lhsT=vproj[:, h * DH:(h + 1) * DH],
                                     rhs=pT[:], start=True, stop=True)
                    copy(t[hh * DH:(hh + 1) * DH, :], a_ps[:],
                         "v" if hh % 2 else "s")
                attnT_sb.append(t)

            o_ps = PS(R, D)
            nc.tensor.matmul(out=o_ps[:], lhsT=attnT_sb[0][:], rhs=W["aq_wo"][0], start=True, stop=False)
            nc.tensor.matmul(out=o_ps[:], lhsT=attnT_sb[1][:], rhs=W["aq_wo"][1], start=False, stop=False)
            tok_bias(o_ps[:], "aq_bo", 256)
            nc.tensor.matmul(out=o_ps[:], lhsT=ident[:R, :R], rhs=traj1[:], start=False, stop=True)
            traj2 = sb(R, D, "traj2")
            layer_norm(o_ps[:], "n1_g", "n1_b", traj2[:], "1")

            # ---------------- ego branch (softmax over 1 key == 1) + LN2 ----
            egoT = transpose256(ego, BL, "egoT")
            ev_ps = PS(BL, 256)
            nc.tensor.matmul(out=ev_ps[:], lhsT=egoT[0][:, :BL], rhs=W["eq_wv"][0], start=True, stop=False)
            nc.tensor.matmul(out=ev_ps[:], lhsT=egoT[1][:, :BL], rhs=W["eq_wv"][1], start=False, stop=False)
            tok_bias(ev_ps[:], "eq_bv", 256, stop=True, rows=BL)
            ev = sb(BL, 256, "ev")
            copy(ev[:], ev_ps[:], "s")
            evT = transpose256(ev, BL, "evT")
            e_ps = PS(BL, 256)
            nc.tensor.matmul(out=e_ps[:], lhsT=evT[0][:, :BL], rhs=W["eq_wo"][0], start=True, stop=False)
            nc.tensor.matmul(out=e_ps[:], lhsT=evT[1][:, :BL], rhs=W["eq_wo"][1], start=False, stop=False)
            tok_bias(e_ps[:], "eq_bo", 256, stop=True, rows=BL)
            eout = sb(BL, 256, "eout")
            copy(eout[:], e_ps[:], "s")
            ebc_ps = PS(R, 256)
            nc.tensor.matmul(out=ebc_ps[:], lhsT=bsel[:, :R], rhs=eout[:],
                             start=True, stop=True)
            t3p = sb(R, D, "t3p")
            nc.vector.tensor_tensor(out=t3p[:], in0=traj2[:], in1=ebc_ps[:], op=OP.add)
            traj3 = sb(R, D, "traj3")
            layer_norm(t3p[:], "n2_g", "n2_b", traj3[:], "2")

            # ---------------- FFN + LN3 (no residual) ----------------
            tr3T = transpose256(traj3, R, "tr3T")
            hT = []
            for f in range(FF // 128):
                h_ps = PS(128, R)
                for i in range(2):
                    nc.tensor.matmul(out=h_ps[:],
                                     lhsT=W["ffn_w1"][i][:, f * 128:(f + 1) * 128],
                                     rhs=tr3T[i][:, :R], start=(i == 0), stop=False)
                nc.tensor.matmul(out=h_ps[:], lhsT=bslice("ffn_b1", f * 128, 128),
                                 rhs=ones[0:1, :R], start=False, stop=True)
                t = ttp.tile([128, R], F32, tag=f"hT{f % 4}_{f // 4}", name=f"hT{f}")
                nc.scalar.activation(out=t[:], in_=h_ps[:], func=AF.Relu)
                hT.append(t)
            f2_ps = PS(R, D)
            for f in range(FF // 128):
                nc.tensor.matmul(out=f2_ps[:], lhsT=hT[f][:], rhs=W["ffn_w2"][f],
                                 start=(f == 0), stop=False)
            tok_bias(f2_ps[:], "ffn_b2", 256, stop=True)
            traj4a = sb(R, D, "traj4a")
            layer_norm(f2_ps[:], "n3_g", "n3_b", traj4a[:], "3")

            # ---------------- FiLM time modulation ----------------
            mish = sb(BL, D, "mish")
            mt = sb(BL, D, "mish_t")
            nc.scalar.activation(out=mt[:], in_=time_e[:], func=AF.Exp)
            mu = sb(BL, D, "mish_u")
            nc.scalar.activation(out=mu[:], in_=mt[:], func=AF.Square, bias=1.0)
            mden = sb(BL, D, "mish_d")
            nc.vector.tensor_scalar_add(out=mden[:], in0=mu[:], scalar1=1.0)
            nc.vector.reciprocal(mden[:], mden[:])
            mnum = sb(BL, D, "mish_n")
            nc.vector.tensor_scalar_add(out=mnum[:], in0=mu[:], scalar1=-1.0)
            nc.vector.tensor_tensor(out=mnum[:], in0=mnum[:], in1=mden[:], op=OP.mult)
            nc.vector.tensor_tensor(out=mish[:], in0=time_e[:], in1=mnum[:], op=OP.mult)
            mishT = transpose256(mish, BL, "mishT")
            ss_ps = PS(BL, 512)
            nc.tensor.matmul(out=ss_ps[:], lhsT=mishT[0][:, :BL], rhs=W["mod_w"][0], start=True, stop=False)
            nc.tensor.matmul(out=ss_ps[:], lhsT=mishT[1][:, :BL], rhs=W["mod_w"][1], start=False, stop=False)
            tok_bias(ss_ps[:], "mod_b", 512, stop=True, rows=BL)
            ss = sb(BL, 512, "ss")
            copy(ss[:], ss_ps[:], "s")
            ssb_ps = PS(R, 512)
            nc.tensor.matmul(out=ssb_ps[:], lhsT=bsel[:, :R], rhs=ss[:],
                             start=True, stop=True)
            tf1 = sb(R, D, "tf1")
            nc.vector.tensor_tensor(out=tf1[:], in0=traj4a[:], in1=ssb_ps[:, :256], op=OP.mult)
            nc.vector.tensor_tensor(out=tf1[:], in0=tf1[:], in1=ssb_ps[:, 256:], op=OP.add)
            traj4 = sb(R, D, "traj4")
            nc.vector.tensor_tensor(out=traj4[:], in0=tf1[:], in1=traj4a[:], op=OP.add)

            # ---------------- heads ----------------
            out_sb = sb(R, 25, "out_sb")
            tr4T = transpose256(traj4, R, "tr4T")

            def linear_relu(in_T, wname, bname, tagbase, relu=True):
                ps = PS(R, D)
                for i in range(2):
                    nc.tensor.matmul(out=ps[:], lhsT=in_T[i][:, :R], rhs=W[wname][i],
                                     start=(i == 0), stop=False)
                tok_bias(ps[:], bname, 256, stop=True)
                t = sb(R, D, tagbase)
                if relu:
                    nc.scalar.activation(out=t[:], in_=ps[:], func=AF.Relu)
                else:
                    copy(t[:], ps[:], "v")
                return t

            # cls branch
            c1r = linear_relu(tr4T, "cls_w1", "cls_b1", "c1r")
            c1 = sb(R, D, "c1")
            layer_norm(c1r[:], "cls_ln1_g", "cls_ln1_b", c1[:], "c1")
            c1T = transpose256(c1, R, "c1T")
            c2r = linear_relu(c1T, "cls_w2", "cls_b2", "c2r")
            c2 = sb(R, D, "c2")
            layer_norm(c2r[:], "cls_ln2_g", "cls_ln2_b", c2[:], "c2")
            c2T = transpose256(c2, R, "c2T")
            cls_ps = PS(R, 1)
            nc.tensor.matmul(out=cls_ps[:], lhsT=c2T[0][:, :R], rhs=W["cls_w3"][0], start=True, stop=False)
            nc.tensor.matmul(out=cls_ps[:], lhsT=c2T[1][:, :R], rhs=W["cls_w3"][1], start=False, stop=False)
            tok_bias(cls_ps[:], "cls_b3", 1, stop=True)
            nc.scalar.copy(out=out_sb[:, 24:25], in_=cls_ps[:])

            # reg branch
            r1 = linear_relu(tr4T, "reg_w1", "reg_b1", "r1")
            r1T = transpose256(r1, R, "r1T")
            r2 = linear_relu(r1T, "reg_w2", "reg_b2", "r2")
            r2T = transpose256(r2, R, "r2T")
            r3_ps = PS(R, 24)
            nc.tensor.matmul(out=r3_ps[:], lhsT=r2T[0][:, :R], rhs=W["reg_w3"][0], start=True, stop=False)
            nc.tensor.matmul(out=r3_ps[:], lhsT=r2T[1][:, :R], rhs=W["reg_w3"][1], start=False, stop=False)
            tok_bias(r3_ps[:], "reg_b3", 24, stop=True)

            r3v = r3_ps[:].rearrange("p (t c) -> p t c", c=3)
            outv = out_sb[:, 0:24].rearrange("p (t c) -> p t c", c=3)
            noisyv = noisy[:].rearrange("p (t c) -> p t c", c=2)
            nc.vector.tensor_tensor(out=outv[:, :, 0:2], in0=r3v[:, :, 0:2],
                                    in1=noisyv, op=OP.add)
            th = sb(R, P, "th")
            nc.scalar.activation(out=th[:], in_=r3v[:, :, 2], func=AF.Tanh)
            nc.vector.tensor_scalar_mul(out=outv[:, :, 2], in0=th[:],
                                        scalar1=float(np.pi))

            nc.sync.dma_start(out=out_h.ap(), in_=out_sb[:])

    nc.compile()
    return nc


# ---------------------------------------------------------------------------
# host side
# ---------------------------------------------------------------------------
_PROG = None


def _get_prog():
    global _PROG
    if _PROG is None:
        _PROG = build_program()
    return _PROG


def host_prep(inputs):
    f32, i32 = np.float32, np.int32
    traj = np.ascontiguousarray(np.asarray(inputs["traj_feature"], f32))
    noisy = np.ascontiguousarray(np.asarray(inputs["noisy_traj_points"], f32))
    bev = np.asarray(inputs["bev_feature"], f32)
    agents = np.ascontiguousarray(np.asarray(inputs["agents_query"], f32))
    ego = np.ascontiguousarray(np.asarray(inputs["ego_query"], f32))
    time_e = np.ascontiguousarray(np.asarray(inputs["time_emb"] if "time_emb" in inputs
                                             else inputs["time_embed"], f32))

    # bilinear corner indices / weights (host: pure index arithmetic)
    g = noisy / BEV_RANGE                      # (B, M, P, 2)
    x = (g[..., 0] + 1.0) * (HW * 0.5) - 0.5
    y = (g[..., 1] + 1.0) * (HW * 0.5) - 0.5
    x0 = np.floor(x).astype(np.int64)
    y0 = np.floor(y).astype(np.int64)
    wx = np.stack([x0 + 1 - x, x - x0], -1)    # (B,M,P,2) weights for x0, x0+1
    wy = np.stack([y0 + 1 - y, y - y0], -1)
    xg = np.clip(x0, 0, HW - 2)
    # slot weights for the gathered x-pair [xg, xg+1]
    wxs = np.zeros(x.shape + (2,), f32)
    for k in (0, 1):
        xi = x0 + k
        valid = (xi >= 0) & (xi <= HW - 1)
        for s in (0, 1):
            wxs[..., s] += np.where(valid & (xi == xg + s), wx[..., k], 0.0)
    # y rows gathered: clip(y0,0,127) and clip(y0+1,0,127), zero-weight if OOB
    ys = np.stack([np.clip(y0, 0, HW - 1), np.clip(y0 + 1, 0, HW - 1)], -1)
    wys = np.stack([np.where((y0 >= 0) & (y0 <= HW - 1), wy[..., 0], 0.0),
                    np.where((y0 + 1 >= 0) & (y0 + 1 <= HW - 1), wy[..., 1], 0.0)], -1)
    # w4[b,m,pt,yc,xc], offs[b,m,pt,yc]
    w4 = (wys[..., :, None] * wxs[..., None, :]).astype(f32)      # (B,M,P,2,2)
    # element offset into per-core NHWC shard: ((b_loc*HW + y)*HW + xg)*D
    b_loc = (np.arange(B) % BL)[:, None, None, None]
    offs = (b_loc * HW + ys) * HW + xg[..., None]                 # pixel index
    offs = offs.astype(i32)

    # agent-attention block mask (same for every core)
    amask = np.zeros((R, NH * AT), f32)
    for p_ in range(R):
        bi = p_ // M
        for h in range(NH):
            amask[p_, h * AT + bi * A: h * AT + bi * A + A] = 1.0

    # bias table / LN table / weights (shared across cores)
    bigb = np.zeros((1, NB), f32)
    for n, s_ in BIAS_SPEC:
        key = {"bev_attn_b": "bev_attn_b", "bev_out_b": "bev_out_b"}.get(n, n)
        bigb[0, BOFF[n]:BOFF[n] + s_] = np.asarray(inputs[key], f32).reshape(-1)
    lnv = np.concatenate([np.asarray(inputs[n], f32).reshape(-1) for n in LN_SPEC])[None]
    bvp = np.asarray(inputs["bev_value_b"], f32).reshape(2, 128).T.copy()  # [128, 2]

    bsel = np.zeros((BL, R), f32)
    for b_ in range(BL):
        bsel[b_, b_ * M:(b_ + 1) * M] = 1.0

    shared = {"bigb": bigb, "lnv": np.ascontiguousarray(lnv), "bv_pack": bvp,
              "amask": amask, "bsel": bsel}
    for n, _k, _n2 in WEIGHTS_2D:
        shared[n] = np.ascontiguousarray(np.asarray(inputs[n], f32))

    in_maps = []
    for c in range(NCORES):
        sl = slice(c * BL, (c + 1) * BL)
        m = dict(shared)
        m["bev"] = np.ascontiguousarray(bev[sl].transpose(0, 2, 3, 1)).reshape(-1, D)
        m["traj"] = traj[sl].reshape(R, D)
        m["agents"] = agents[sl].reshape(AT, D)
        m["ego"] = ego[sl].reshape(BL, D)
        m["time_emb"] = time_e[sl].reshape(BL, D)
        m["noisy"] = noisy[sl].reshape(R, P * 2)
        m["offs"] = offs[sl].reshape(R, P * 2)
        m["w4"] = w4[sl].reshape(R, P * 4)
        in_maps.append(m)
    return in_maps


def kernel(**inputs):
    nc = _get_prog()
    in_maps = host_prep(inputs)
    res = bass_utils.run_bass_kernel_spmd(nc, in_maps, core_ids=list(range(NCORES)))
    outs = [np.asarray(r["out"]) for r in res.results]
    poses = np.concatenate([o[:, :24].reshape(BL, M, P, 3) for o in outs], axis=0)
    cls = np.concatenate([o[:, 24].reshape(BL, M) for o in outs], axis=0)
    return poses.astype(np.float32), cls.astype(np.float32)


# revision 14
# speedup vs baseline: 1.0596x; 1.0596x over previous
"""Trainium2 Bass kernel for nn_CustomTransformerDecoderLayer_3753801416834.

Strategy (pure data-parallel over batch, 4 batches per core on 8 cores):
  * grid_sample commutes with the 1x1 value conv (both linear) -> gather only
    the 2x2 bilinear corner patches from bev_feature and project afterwards.
    Host passes bev in NHWC layout so one gather descriptor is 512 contiguous
    floats (2 x-adjacent pixels x 256 channels) -> 1280 descriptors/core.
  * ego attention has Lk=1 so softmax == 1: collapses to a broadcast linear.
  * point-attention weights fold into the bilinear corner weights (one 32-way
    weighted reduce per query), and bias-terms fold into K=1 matmuls.
  * agent attention is batched across 4 batches x 8 heads as block-diagonal
    [80 x 128] score matrices with a multiplicative 0/1 mask.
Host-side work is limited to sharding, layout permutation and integer corner
index/weight prep; all tensor math runs on-device.
"""

import sys

import numpy as np

for _p in ("/opt/trn_rl_repo",):
    if _p not in sys.path:
        sys.path.insert(0, _p)

import concourse.bass as bass  # noqa: E402
import concourse.mybir as mybir  # noqa: E402
import concourse.tile as tile  # noqa: E402
from concourse import bacc, bass_utils  # noqa: E402
from concourse.masks import make_identity  # noqa: E402

F32 = mybir.dt.float32
I32 = mybir.dt.int32
AX = mybir.AxisListType.X
OP = mybir.AluOpType
AF = mybir.ActivationFunctionType

NCORES = 8
B, M, P, D, A, FF = 32, 20, 8, 256, 32, 1024
BL = B // NCORES          # local batches per core = 4
R = BL * M                # token rows per core = 80
AT = BL * A               # agent tokens per core = 128
NH, DH = 8, 32            # heads, head dim
HW = 128                  # bev spatial size
BEV_RANGE = 32.0
SCALE = 1.0 / float(np.sqrt(DH))

# ---- bias table layout (host & device share) -------------------------------
BIAS_SPEC = [
    ("aq_bq", 256), ("aq_bk", 256), ("aq_bv", 256), ("aq_bo", 256),
    ("eq_bv", 256), ("eq_bo", 256),
    ("ffn_b1", 1024), ("ffn_b2", 256),
    ("mod_b", 512),
    ("cls_b1", 256), ("cls_b2", 256), ("cls_b3", 1),
    ("reg_b1", 256), ("reg_b2", 256), ("reg_b3", 24),
    ("bev_attn_b", 8), ("bev_out_b", 256),
]
BOFF = {}
_o = 0
for _n, _s in BIAS_SPEC:
    BOFF[_n] = _o
    _o += _s
NB = _o

LN_SPEC = ["n1_g", "n1_b", "n2_g", "n2_b", "n3_g", "n3_b",
           "cls_ln1_g", "cls_ln1_b", "cls_ln2_g", "cls_ln2_b"]
LOFF = {n: i * 256 for i, n in enumerate(LN_SPEC)}
NL = 256 * len(LN_SPEC)

WEIGHTS_2D = [  # name -> (K, N); loaded as K/128 chunks of [128, N]
    ("aq_wq", 256, 256), ("aq_wk", 256, 256), ("aq_wv", 256, 256),
    ("aq_wo", 256, 256), ("eq_wv", 256, 256), ("eq_wo", 256, 256),
    ("ffn_w1", 256, 1024), ("ffn_w2", 1024, 256), ("mod_w", 256, 512),
    ("cls_w1", 256, 256), ("cls_w2", 256, 256), ("cls_w3", 256, 1),
    ("reg_w1", 256, 256), ("reg_w2", 256, 256), ("reg_w3", 256, 24),
    ("bev_value_w", 256, 256), ("bev_out_w", 256, 256), ("bev_attn_w", 256, 8),
]


def build_program(enable_asserts: bool = False):
    nc = bacc.Bacc(
        "TRN2",
        target_bir_lowering=False,
        debug=False,
        enable_asserts=enable_asserts,
        num_devices=NCORES,
    )

    def din(name, shape, dtype=F32):
        return nc.dram_tensor(name, list(shape), dtype, kind="ExternalInput")

    bev_h = din("bev", (BL * HW * HW, D))
    traj_h = din("traj", (R, D))
    agents_h = din("agents", (AT, D))
    ego_h = din("ego", (BL, D))
    time_h = din("time_emb", (BL, D))
    noisy_h = din("noisy", (R, P * 2))
    offs_h = din("offs", (R, P * 2), I32)
    w4_h = din("w4", (R, P * 4))
    amask_h = din("amask", (R, NH * AT))
    bsel_h = din("bsel", (BL, R))
    bigb_h = din("bigb", (1, NB))
    lnv_h = din("lnv", (1, NL))
    bvp_h = din("bv_pack", (128, 2))
    w_h = {n: din(n, (k, nn)) for n, k, nn in WEIGHTS_2D}
    out_h = nc.dram_tensor("out", [R, 25], F32, kind="ExternalOutput")

    with tile.TileContext(nc) as tc:
        with (
            tc.tile_pool(name="w", bufs=1) as wp,
            tc.tile_pool(name="act", bufs=1) as ap_,
            tc.tile_pool(name="tmp", bufs=4) as tp,
            tc.tile_pool(name="tt", bufs=1) as ttp,
            tc.tile_pool(name="ps", bufs=8, space="PSUM") as pp,
        ):
            # ---------------- constants / weights ----------------
            ident = wp.tile([128, 128], F32, tag="ident")
            make_identity(nc, ident[:])
            ones = wp.tile([1, 128], F32, tag="ones")
            nc.gpsimd.memset(ones[:], 1.0)
            epsc = wp.tile([128, 1], F32, tag="epsc")
            nc.gpsimd.memset(epsc[:], 1e-5)

            W = {}
            for wi, (name, K, N) in enumerate(WEIGHTS_2D):
                nchunk = K // 128
                t = wp.tile([128, nchunk * N], F32, tag=f"w_{name}", name=f"w_{name}")
                eng = nc.sync if wi % 2 == 0 else nc.scalar
                eng.dma_start(
                    out=t[:].rearrange("p (c n) -> p c n", c=nchunk),
                    in_=w_h[name].ap().rearrange("(c p) n -> p c n", p=128))
                W[name] = [t[:, i * N:(i + 1) * N] for i in range(nchunk)]

            bigb = wp.tile([1, NB], F32, tag="bigb")
            nc.sync.dma_start(out=bigb[:], in_=bigb_h.ap())
            bvp = wp.tile([128, 2], F32, tag="bvp")
            nc.sync.dma_start(out=bvp[:], in_=bvp_h.ap())

            def bslice(name, lo=0, size=None):
                off = BOFF[name] + lo
                if size is None:
                    size = dict((n, s) for n, s in BIAS_SPEC)[name]
                return bigb[0:1, off:off + size]

            # LN vectors replicated across partitions by a broadcast DMA
            lnr = wp.tile([128, NL], F32, tag="lnr")
            nc.scalar.dma_start(out=lnr[:], in_=lnv_h.ap().partition_broadcast(128))

            def lslice(name):
                return lnr[:R, LOFF[name]:LOFF[name] + 256]


            # ---------------- inputs ----------------
            _ldc = [0]
            def load(h, shape, dtype=F32, tag=None):
                t = ap_.tile(list(shape), dtype, tag=tag or h.name, name=tag or h.name)
                eng = nc.sync if _ldc[0] % 2 == 0 else nc.scalar
                _ldc[0] += 1
                eng.dma_start(out=t[:], in_=h.ap())
                return t

            traj = load(traj_h, (R, D))
            agents = load(agents_h, (AT, D))
            ego = load(ego_h, (BL, D))
            time_e = load(time_h, (BL, D))
            noisy = load(noisy_h, (R, P * 2))
            offs = load(offs_h, (R, P * 2), I32)
            w4 = load(w4_h, (R, P * 4))
            amask = load(amask_h, (R, NH * AT))
            bsel = load(bsel_h, (BL, R))

            # gathered 2x2xC corner patches: per (query,point,ycorner) 512 floats
            # HW indirect DMA consumes ONE index per partition, gathering the
            # out-row free size contiguously -> one call per (point, ycorner).
            G = ap_.tile([R, P * 2 * 512], F32, tag="G")
            for k in range(P * 2):
                nc.gpsimd.indirect_dma_start(
                    out=G[:, k * 512:(k + 1) * 512], out_offset=None,
                    in_=bev_h.ap(),
                    in_offset=bass.IndirectOffsetOnAxis(ap=offs[:, k:k + 1], axis=0),
                )

            # ---------------- helpers ----------------
            def PS(p, f):
                return pp.tile([p, f], F32, tag="ps", name="ps")

            def sb(p, f, tag):
                return ap_.tile([p, f], F32, tag=tag, name=tag)

            def copy(dst_ap, src_ap, eng="v"):
                if eng == "v":
                    nc.vector.tensor_copy(out=dst_ap, in_=src_ap)
                else:
                    nc.scalar.copy(out=dst_ap, in_=src_ap)

            def transpose_to(src_ap, pdim, fdim, tag, eng="v"):
                """src [pdim, fdim] (sbuf) -> new sbuf tile [fdim, pdim]."""
                ps = PS(fdim, pdim)
                nc.tensor.transpose(out=ps[:], in_=src_ap, identity=ident[:pdim, :pdim])
                t = ttp.tile([fdim, pdim], F32, tag=tag, name=tag)
                copy(t[:], ps[:], eng)
                return t

            def transpose256(src, pdim, tagbase, eng="v"):
                return [transpose_to(src[:, i * 128:(i + 1) * 128], pdim, 128,
                                     f"{tagbase}{i}", eng) for i in range(2)]

            def tok_bias(ps_ap, name, n, start=False, stop=False, rows=R):
                """out[m, :] += bias (K=1 matmul, ones as lhsT)."""
                nc.tensor.matmul(out=ps_ap, lhsT=ones[0:1, :rows],
                                 rhs=bslice(name, 0, n), start=start, stop=stop)

            def layer_norm(src_ap, gname, bname, dst_ap, sfx):
                srow = sb(R, 1, f"ln_s{sfx}")
                nc.vector.reduce_sum(srow[:], src_ap, axis=AX)
                nm = sb(R, 1, f"ln_nm{sfx}")
                nc.scalar.activation(out=nm[:], in_=srow[:], func=AF.Copy,
                                     scale=-1.0 / D)
                xm = sb(R, D, f"ln_xm{sfx}")
                nc.vector.tensor_scalar_add(out=xm[:], in0=src_ap, scalar1=nm[:])
                sq = tp.tile([R, D], F32, tag="fmatmp", name="fmatmp")
                ssq = sb(R, 1, f"ln_q{sfx}")
                nc.scalar.activation(out=sq[:], in_=xm[:], func=AF.Square,
                                     accum_out=ssq[:])
                # rsqrt(var+eps) on DVE only: quake seed + 2 Newton steps
                v_ = sb(R, 1, f"ln_v{sfx}")
                nc.vector.tensor_scalar(out=v_[:], in0=ssq[:], scalar1=1.0 / D,
                                        scalar2=1e-5, op0=OP.mult, op1=OP.add)
                iv = sb(R, 1, f"ln_iv{sfx}")
                I32v = v_[:].bitcast(I32)
                nc.vector.tensor_scalar(out=iv[:].bitcast(I32), in0=I32v,
                                        scalar1=1, scalar2=None,
                                        op0=OP.arith_shift_right)
                rstd = sb(R, 1, f"ln_r{sfx}")
                nc.vector.tensor_scalar(out=rstd[:].bitcast(I32),
                                        in0=iv[:].bitcast(I32),
                                        scalar1=-1, scalar2=0x5f3759df,
                                        op0=OP.mult, op1=OP.add)
                for it_ in range(2):
                    yy = sb(R, 1, f"ln_y{sfx}{it_}")
                    nc.vector.tensor_tensor(out=yy[:], in0=rstd[:], in1=rstd[:],
                                            op=OP.mult)
                    nc.vector.tensor_tensor(out=yy[:], in0=yy[:], in1=v_[:],
                                            op=OP.mult)
                    nc.vector.tensor_scalar(out=yy[:], in0=yy[:], scalar1=-0.5,
                                            scalar2=1.5, op0=OP.mult, op1=OP.add)
                    nc.vector.tensor_tensor(out=rstd[:], in0=rstd[:], in1=yy[:],
                                            op=OP.mult)
                nc.vector.tensor_scalar_mul(out=xm[:], in0=xm[:], scalar1=rstd[:])
                nc.vector.tensor_tensor(out=dst_ap, in0=xm[:], in1=lslice(gname),
                                        op=OP.mult)
                nc.vector.tensor_tensor(out=dst_ap, in0=dst_ap, in1=lslice(bname),
                                        op=OP.add)

            # ---------------- BEV branch ----------------
            # point-attention weights ptw = softmax(traj @ bev_attn_w + b)
            trajT = transpose256(traj, R, "trajT")
            pw_ps = PS(R, P)
            nc.tensor.matmul(out=pw_ps[:], lhsT=trajT[0][:, :R], rhs=W["bev_attn_w"][0],
                             start=True, stop=False)
            nc.tensor.matmul(out=pw_ps[:], lhsT=trajT[1][:, :R], rhs=W["bev_attn_w"][1],
                             start=False, stop=False)
            tok_bias(pw_ps[:], "bev_attn_b", P, stop=True)
            nmax = sb(R, 1, "pw_nmax")
            nc.vector.reduce_max(nmax[:], pw_ps[:], axis=AX, negate=True)
            pexp = sb(R, P, "pw_exp")
            nc.scalar.activation(out=pexp[:], in_=pw_ps[:], func=AF.Exp,
                                 bias=nmax[:], scale=1.0)
            psum_ = sb(R, 1, "pw_sum")
            nc.vector.reduce_sum(psum_[:], pexp[:], axis=AX)
            prec = sb(R, 1, "pw_rec")
            nc.vector.reciprocal(prec[:], psum_[:])
            ptw = sb(R, P, "ptw")
            nc.vector.tensor_scalar_mul(out=ptw[:], in0=pexp[:], scalar1=prec[:])

            # combined corner weights cw[m, pt*4 + yc*2 + xc]
            cw = sb(R, P * 4, "cw")
            for p_ in range(P):
                nc.vector.tensor_scalar_mul(out=cw[:, 4 * p_:4 * p_ + 4],
                                            in0=w4[:, 4 * p_:4 * p_ + 4],
                                            scalar1=ptw[:, p_:p_ + 1])

            # gq[m, c] = sum_j cw[m, j] * G[m, j, c]   (4 parallel chains)
            accs = []
            for k in range(4):
                a = sb(R, D, f"gacc{k}")
                for ji, j in enumerate(range(k * 8, k * 8 + 8)):
                    pt, rem = divmod(j, 4)
                    yc, xc = divmod(rem, 2)
                    off = (pt * 2 + yc) * 512 + xc * 256
                    gsl = G[:, off:off + 256]
                    if ji == 0:
                        nc.scalar.activation(out=a[:], in_=gsl, func=AF.Copy,
                                             scale=cw[:, j:j + 1])
                    else:
                        t = tp.tile([R, D], F32, tag="fmatmp", name="fmatmp")
                        nc.scalar.activation(out=t[:], in_=gsl, func=AF.Copy,
                                             scale=cw[:, j:j + 1])
                        nc.vector.tensor_tensor(out=a[:], in0=a[:], in1=t[:], op=OP.add)
                accs.append(a)
            nc.vector.tensor_tensor(out=accs[0][:], in0=accs[0][:], in1=accs[1][:], op=OP.add)
            nc.vector.tensor_tensor(out=accs[2][:], in0=accs[2][:], in1=accs[3][:], op=OP.add)
            gq = sb(R, D, "gq")
            nc.vector.tensor_tensor(out=gq[:], in0=accs[0][:], in1=accs[2][:], op=OP.add)

            # s[m] = sum_j cw[m, j]  (validity-weighted bias scale), as [1, R]
            srow = sb(R, 1, "cw_s")
            nc.vector.reduce_sum(srow[:], cw[:], axis=AX)
            sT_ps = PS(1, R)
            nc.tensor.transpose(out=sT_ps[:], in_=srow[:, 0:1], identity=ident[:R, :R])
            sT = sb(1, R, "cw_sT")
            copy(sT[:], sT_ps[:], "s")

            # W2 = bev_value_w @ bev_out_w  (on device, one-time)
            WvT = []  # e-chunks [128e, 256c]
            for j in range(2):
                t = wp.tile([128, 256], F32, tag=f"WvT{j}", name=f"WvT{j}")
                for i in range(2):
                    ps = PS(128, 128)
                    nc.tensor.transpose(out=ps[:],
                                        in_=W["bev_value_w"][i][:, j * 128:(j + 1) * 128],
                                        identity=ident[:])
                    copy(t[:, i * 128:(i + 1) * 128], ps[:], "s")
                WvT.append(t)
            W2 = []
            for i in range(2):
                ps = PS(128, 256)
                for j in range(2):
                    nc.tensor.matmul(out=ps[:], lhsT=WvT[j][:, i * 128:(i + 1) * 128],
                                     rhs=W["bev_out_w"][j],
                                     start=(j == 0), stop=(j == 1))
                t = wp.tile([128, 256], F32, tag=f"W2_{i}", name=f"W2_{i}")
                copy(t[:], ps[:], "v")
                W2.append(t)

            # bvWo = bev_value_b @ bev_out_w  [1, 256]
            bw_ps = PS(1, 256)
            nc.tensor.matmul(out=bw_ps[:], lhsT=bvp[:, 0:1], rhs=W["bev_out_w"][0],
                             start=True, stop=False)
            nc.tensor.matmul(out=bw_ps[:], lhsT=bvp[:, 1:2], rhs=W["bev_out_w"][1],
                             start=False, stop=True)
            bvWo = sb(1, 256, "bvWo")
            copy(bvWo[:], bw_ps[:], "s")

            # traj1 = traj + gq @ W2 + s * bvWo + bev_out_b
            gqT = transpose256(gq, R, "gqT")
            t1_ps = PS(R, D)
            nc.tensor.matmul(out=t1_ps[:], lhsT=gqT[0][:, :R], rhs=W2[0][:], start=True, stop=False)
            nc.tensor.matmul(out=t1_ps[:], lhsT=gqT[1][:, :R], rhs=W2[1][:], start=False, stop=False)
            nc.tensor.matmul(out=t1_ps[:], lhsT=sT[0:1, :R], rhs=bvWo[0:1, :], start=False, stop=False)
            tok_bias(t1_ps[:], "bev_out_b", 256)
            nc.tensor.matmul(out=t1_ps[:], lhsT=ident[:R, :R], rhs=traj[:], start=False, stop=True)
            traj1 = sb(R, D, "traj1")
            copy(traj1[:], t1_ps[:], "v")

            # ---------------- agents cross-attention + LN1 ----------------
            agT = transpose256(agents, AT, "agT")
            tr1T = transpose256(traj1, R, "tr1T")

            def projT(wname, bname, rhs_tiles, ntok, tagbase):
                """x @ W + b, produced transposed: 4 chunks [64 dout, ntok]
                so per-head slices start at partition base 0 or 32."""
                outs = []
                for j in range(4):
                    ps = PS(64, ntok)
                    for i in range(2):
                        nc.tensor.matmul(out=ps[:],
                                         lhsT=W[wname][i][:, j * 64:(j + 1) * 64],
                                         rhs=rhs_tiles[i][:, :ntok],
                                         start=(i == 0), stop=False)
                    nc.tensor.matmul(out=ps[:], lhsT=bslice(bname, j * 64, 64),
                                     rhs=ones[0:1, :ntok], start=False, stop=True)
                    t = ttp.tile([64, ntok], F32, tag=f"{tagbase}{j}", name=f"{tagbase}{j}")
                    copy(t[:], ps[:], "s" if j % 2 else "v")
                    outs.append(t)
                return outs

            qT = projT("aq_wq", "aq_bq", tr1T, R, "qT")
            kT = projT("aq_wk", "aq_bk", agT, AT, "kT")

            # v_proj [AT, 256]
            v_ps = PS(AT, 256)
            nc.tensor.matmul(out=v_ps[:], lhsT=agT[0][:], rhs=W["aq_wv"][0], start=True, stop=False)
            nc.tensor.matmul(out=v_ps[:], lhsT=agT[1][:], rhs=W["aq_wv"][1], start=False, stop=False)
            tok_bias(v_ps[:], "aq_bv", 256, stop=True, rows=AT)
            vproj = sb(AT, 256, "vproj")
            copy(vproj[:], v_ps[:], "v")

            # scores -> masked softmax (no max-sub needed; values bounded)
            eall = sb(R, NH * AT, "eall")
            for h in range(NH):
                j, r0 = divmod(h, 2)
                r0 *= DH
                s_ps = PS(R, AT)
                nc.tensor.matmul(out=s_ps[:], lhsT=qT[j][r0:r0 + DH, :R],
                                 rhs=kT[j][r0:r0 + DH, :AT], start=True, stop=True)
                nc.scalar.activation(out=eall[:, h * AT:(h + 1) * AT], in_=s_ps[:],
                                     func=AF.Exp, scale=SCALE)
            nc.vector.tensor_tensor(out=eall[:], in0=eall[:], in1=amask[:], op=OP.mult)
            rs = sb(R, NH, "rs")
            nc.vector.reduce_sum(rs[:], eall[:].rearrange("p (h k) -> p h k", k=AT), axis=AX)
            rin = sb(R, NH, "rin")
            nc.vector.reciprocal(rin[:], rs[:])
            pn = sb(R, NH * AT, "pn")
            for h in range(NH):
                nc.vector.tensor_scalar_mul(out=pn[:, h * AT:(h + 1) * AT],
                                            in0=eall[:, h * AT:(h + 1) * AT],
                                            scalar1=rin[:, h:h + 1])

            attnT_sb = []
            for j in range(2):
                t = ttp.tile([128, R], F32, tag=f"attnT{j}", name=f"attnT{j}")
                for hh in range(4):
                    h = j * 4 + hh
                    pT_ps = PS(AT, R)
                    nc.tensor.transpose(out=pT_ps[:], in_=pn[:, h * AT:(h + 1) * AT],
                                        identity=ident[:R, :R])
                    pT = tp.tile([AT, R], F32, tag="pT", name="pT")
                    copy(pT[:], pT_ps[:], "s" if hh % 2 else "v")
                    a_ps = PS(DH, R)
                    nc.tensor.matmul(out=a_ps[:],
                                     lhsT=vproj[:, h * DH:(h + 1) * DH],
                                     rhs=pT[:], start=True, stop=True)
                    copy(t[hh * DH:(hh + 1) * DH, :], a_ps[:],
                         "v" if hh % 2 else "s")
                attnT_sb.append(t)

            o_ps = PS(R, D)
            nc.tensor.matmul(out=o_ps[:], lhsT=attnT_sb[0][:], rhs=W["aq_wo"][0], start=True, stop=False)
            nc.tensor.matmul(out=o_ps[:], lhsT=attnT_sb[1][:], rhs=W["aq_wo"][1], start=False, stop=False)
            tok_bias(o_ps[:], "aq_bo", 256)
            nc.tensor.matmul(out=o_ps[:], lhsT=ident[:R, :R], rhs=traj1[:], start=False, stop=True)
            traj2 = sb(R, D, "traj2")
            layer_norm(o_ps[:], "n1_g", "n1_b", traj2[:], "1")

            # ---------------- ego branch (softmax over 1 key == 1) + LN2 ----
            egoT = transpose256(ego, BL, "egoT")
            ev_ps = PS(BL, 256)
            nc.tensor.matmul(out=ev_ps[:], lhsT=egoT[0][:, :BL], rhs=W["eq_wv"][0], start=True, stop=False)
            nc.tensor.matmul(out=ev_ps[:], lhsT=egoT[1][:, :BL], rhs=W["eq_wv"][1], start=False, stop=False)
            tok_bias(ev_ps[:], "eq_bv", 256, stop=True, rows=BL)
            ev = sb(BL, 256, "ev")
            copy(ev[:], ev_ps[:], "s")
            evT = transpose256(ev, BL, "evT")
            e_ps = PS(BL, 256)
            nc.tensor.matmul(out=e_ps[:], lhsT=evT[0][:, :BL], rhs=W["eq_wo"][0], start=True, stop=False)
            nc.tensor.matmul(out=e_ps[:], lhsT=evT[1][:, :BL], rhs=W["eq_wo"][1], start=False, stop=False)
            tok_bias(e_ps[:], "eq_bo", 256, stop=True, rows=BL)
            eout = sb(BL, 256, "eout")
            copy(eout[:], e_ps[:], "s")
            ebc_ps = PS(R, 256)
            nc.tensor.matmul(out=ebc_ps[:], lhsT=bsel[:, :R], rhs=eout[:],
                             start=True, stop=True)
            t3p = sb(R, D, "t3p")
            nc.vector.tensor_tensor(out=t3p[:], in0=traj2[:], in1=ebc_ps[:], op=OP.add)
            traj3 = sb(R, D, "traj3")
            layer_norm(t3p[:], "n2_g", "n2_b", traj3[:], "2")

            # ---------------- FFN + LN3 (no residual) ----------------
            tr3T = transpose256(traj3, R, "tr3T")
            hT = []
            for f in range(FF // 128):
                h_ps = PS(128, R)
                for i in range(2):
                    nc.tensor.matmul(out=h_ps[:],
                                     lhsT=W["ffn_w1"][i][:, f * 128:(f + 1) * 128],
                                     rhs=tr3T[i][:, :R], start=(i == 0), stop=False)
                nc.tensor.matmul(out=h_ps[:], lhsT=bslice("ffn_b1", f * 128, 128),
                                 rhs=ones[0:1, :R], start=False, stop=True)
                t = ttp.tile([128, R], F32, tag=f"hT{f % 4}_{f // 4}", name=f"hT{f}")
                nc.scalar.activation(out=t[:], in_=h_ps[:], func=AF.Relu)
                hT.append(t)
            f2_ps = PS(R, D)
            for f in range(FF // 128):
                nc.tensor.matmul(out=f2_ps[:], lhsT=hT[f][:], rhs=W["ffn_w2"][f],
                                 start=(f == 0), stop=False)
            tok_bias(f2_ps[:], "ffn_b2", 256, stop=True)
            traj4a = sb(R, D, "traj4a")
            layer_norm(f2_ps[:], "n3_g", "n3_b", traj4a[:], "3")

            # ---------------- FiLM time modulation ----------------
            mish = sb(BL, D, "mish")
            mt = sb(BL, D, "mish_t")
            nc.scalar.activation(out=mt[:], in_=time_e[:], func=AF.Exp)
            mu = sb(BL, D, "mish_u")
            nc.scalar.activation(out=mu[:], in_=mt[:], func=AF.Square, bias=1.0)
            mden = sb(BL, D, "mish_d")
            nc.vector.tensor_scalar_add(out=mden[:], in0=mu[:], scalar1=1.0)
            nc.vector.reciprocal(mden[:], mden[:])
            mnum = sb(BL, D, "mish_n")
            nc.vector.tensor_scalar_add(out=mnum[:], in0=mu[:], scalar1=-1.0)
            nc.vector.tensor_tensor(out=mnum[:], in0=mnum[:], in1=mden[:], op=OP.mult)
            nc.vector.tensor_tensor(out=mish[:], in0=time_e[:], in1=mnum[:], op=OP.mult)
            mishT = transpose256(mish, BL, "mishT")
            ss_ps = PS(BL, 512)
            nc.tensor.matmul(out=ss_ps[:], lhsT=mishT[0][:, :BL], rhs=W["mod_w"][0], start=True, stop=False)
            nc.tensor.matmul(out=ss_ps[:], lhsT=mishT[1][:, :BL], rhs=W["mod_w"][1], start=False, stop=False)
            tok_bias(ss_ps[:], "mod_b", 512, stop=True, rows=BL)
            ss = sb(BL, 512, "ss")
            copy(ss[:], ss_ps[:], "s")
            ssb_ps = PS(R, 512)
            nc.tensor.matmul(out=ssb_ps[:], lhsT=bsel[:, :R], rhs=ss[:],
                             start=True, stop=True)
            tf1 = sb(R, D, "tf1")
            nc.vector.tensor_tensor(out=tf1[:], in0=traj4a[:], in1=ssb_ps[:, :256], op=OP.mult)
            nc.vector.tensor_tensor(out=tf1[:], in0=tf1[:], in1=ssb_ps[:, 256:], op=OP.add)
            traj4 = sb(R, D, "traj4")
            nc.vector.tensor_tensor(out=traj4[:], in0=tf1[:], in1=traj4a[:], op=OP.add)

            # ---------------- heads ----------------
            out_sb = sb(R, 25, "out_sb")
            tr4T = transpose256(traj4, R, "tr4T")

            def linear_relu(in_T, wname, bname, tagbase, relu=True):
                ps = PS(R, D)
                for i in range(2):
                    nc.tensor.matmul(out=ps[:], lhsT=in_T[i][:, :R], rhs=W[wname][i],
                                     start=(i == 0), stop=False)
                tok_bias(ps[:], bname, 256, stop=True)
                t = sb(R, D, tagbase)
                if relu:
                    nc.scalar.activation(out=t[:], in_=ps[:], func=AF.Relu)
                else:
                    copy(t[:], ps[:], "v")
                return t

            # cls branch
            c1r = linear_relu(tr4T, "cls_w1", "cls_b1", "c1r")
            c1 = sb(R, D, "c1")
            layer_norm(c1r[:], "cls_ln1_g", "cls_ln1_b", c1[:], "c1")
            c1T = transpose256(c1, R, "c1T")
            c2r = linear_relu(c1T, "cls_w2", "cls_b2", "c2r")
            c2 = sb(R, D, "c2")
            layer_norm(c2r[:], "cls_ln2_g", "cls_ln2_b", c2[:], "c2")
            c2T = transpose256(c2, R, "c2T")
            cls_ps = PS(R, 1)
            nc.tensor.matmul(out=cls_ps[:], lhsT=c2T[0][:, :R], rhs=W["cls_w3"][0], start=True, stop=False)
            nc.tensor.matmul(out=cls_ps[:], lhsT=c2T[1][:, :R], rhs=W["cls_w3"][1], start=False, stop=False)
            tok_bias(cls_ps[:], "cls_b3", 1, stop=True)
            nc.scalar.copy(out=out_sb[:, 24:25], in_=cls_ps[:])

            # reg branch
            r1 = linear_relu(tr4T, "reg_w1", "reg_b1", "r1")
            r1T = transpose256(r1, R, "r1T")
            r2 = linear_relu(r1T, "reg_w2", "reg_b2", "r2")
            r2T = transpose256(r2, R, "r2T")
            r3_ps = PS(R, 24)
            nc.tensor.matmul(out=r3_ps[:], lhsT=r2T[0][:, :R], rhs=W["reg_w3"][0], start=True, stop=False)
            nc.tensor.matmul(out=r3_ps[:], lhsT=r2T[1][:, :R], rhs=W["reg_w3"][1], start=False, stop=False)
            tok_bias(r3_ps[:], "reg_b3", 24, stop=True)

            r3v = r3_ps[:].rearrange("p (t c) -> p t c", c=3)
            outv = out_sb[:, 0:24].rearrange("p (t c) -> p t c", c=3)
            noisyv = noisy[:].rearrange("p (t c) -> p t c", c=2)
            nc.vector.tensor_tensor(out=outv[:, :, 0:2], in0=r3v[:, :, 0:2],
                                    in1=noisyv, op=OP.add)
            th = sb(R, P, "th")
            nc.scalar.activation(out=th[:], in_=r3v[:, :, 2], func=AF.Tanh)
            nc.vector.tensor_scalar_mul(out=outv[:, :, 2], in0=th[:],
                                        scalar1=float(np.pi))

            nc.sync.dma_start(out=out_h.ap(), in_=out_sb[:])

    nc.compile()
    return nc


# ---------------------------------------------------------------------------
# host side
# ---------------------------------------------------------------------------
_PROG = None


def _get_prog():
    global _PROG
    if _PROG is None:
        _PROG = build_program()
    return _PROG


def host_prep(inputs):
    f32, i32 = np.float32, np.int32
    traj = np.ascontiguousarray(np.asarray(inputs["traj_feature"], f32))
    noisy = np.ascontiguousarray(np.asarray(inputs["noisy_traj_points"], f32))
    bev = np.asarray(inputs["bev_feature"], f32)
    agents = np.ascontiguousarray(np.asarray(inputs["agents_query"], f32))
    ego = np.ascontiguousarray(np.asarray(inputs["ego_query"], f32))
    time_e = np.ascontiguousarray(np.asarray(inputs["time_emb"] if "time_emb" in inputs
                                             else inputs["time_embed"], f32))

    # bilinear corner indices / weights (host: pure index arithmetic)
    g = noisy / BEV_RANGE                      # (B, M, P, 2)
    x = (g[..., 0] + 1.0) * (HW * 0.5) - 0.5
    y = (g[..., 1] + 1.0) * (HW * 0.5) - 0.5
    x0 = np.floor(x).astype(np.int64)
    y0 = np.floor(y).astype(np.int64)
    wx = np.stack([x0 + 1 - x, x - x0], -1)    # (B,M,P,2) weights for x0, x0+1
    wy = np.stack([y0 + 1 - y, y - y0], -1)
    xg = np.clip(x0, 0, HW - 2)
    # slot weights for the gathered x-pair [xg, xg+1]
    wxs = np.zeros(x.shape + (2,), f32)
    for k in (0, 1):
        xi = x0 + k
        valid = (xi >= 0) & (xi <= HW - 1)
        for s in (0, 1):
            wxs[..., s] += np.where(valid & (xi == xg + s), wx[..., k], 0.0)
    # y rows gathered: clip(y0,0,127) and clip(y0+1,0,127), zero-weight if OOB
    ys = np.stack([np.clip(y0, 0, HW - 1), np.clip(y0 + 1, 0, HW - 1)], -1)
    wys = np.stack([np.where((y0 >= 0) & (y0 <= HW - 1), wy[..., 0], 0.0),
                    np.where((y0 + 1 >= 0) & (y0 + 1 <= HW - 1), wy[..., 1], 0.0)], -1)
    # w4[b,m,pt,yc,xc], offs[b,m,pt,yc]
    w4 = (wys[..., :, None] * wxs[..., None, :]).astype(f32)      # (B,M,P,2,2)
    # element offset into per-core NHWC shard: ((b_loc*HW + y)*HW + xg)*D
    b_loc = (np.arange(B) % BL)[:, None, None, None]
    offs = (b_loc * HW + ys) * HW + xg[..., None]                 # pixel index
    offs = offs.astype(i32)

    # agent-attention block mask (same for every core)
    amask = np.zeros((R, NH * AT), f32)
    for p_ in range(R):
        bi = p_ // M
        for h in range(NH):
            amask[p_, h * AT + bi * A: h * AT + bi * A + A] = 1.0

    # bias table / LN table / weights (shared across cores)
    bigb = np.zeros((1, NB), f32)
    for n, s_ in BIAS_SPEC:
        key = {"bev_attn_b": "bev_attn_b", "bev_out_b": "bev_out_b"}.get(n, n)
        bigb[0, BOFF[n]:BOFF[n] + s_] = np.asarray(inputs[key], f32).reshape(-1)
    lnv = np.concatenate([np.asarray(inputs[n], f32).reshape(-1) for n in LN_SPEC])[None]
    bvp = np.asarray(inputs["bev_value_b"], f32).reshape(2, 128).T.copy()  # [128, 2]

    bsel = np.zeros((BL, R), f32)
    for b_ in range(BL):
        bsel[b_, b_ * M:(b_ + 1) * M] = 1.0

    shared = {"bigb": bigb, "lnv": np.ascontiguousarray(lnv), "bv_pack": bvp,
              "amask": amask, "bsel": bsel}
    for n, _k, _n2 in WEIGHTS_2D:
        shared[n] = np.ascontiguousarray(np.asarray(inputs[n], f32))

    in_maps = []
    for c in range(NCORES):
        sl = slice(c * BL, (c + 1) * BL)
        m = dict(shared)
        m["bev"] = np.ascontiguousarray(bev[sl].transpose(0, 2, 3, 1)).reshape(-1, D)
        m["traj"] = traj[sl].reshape(R, D)
        m["agents"] = agents[sl].reshape(AT, D)
        m["ego"] = ego[sl].reshape(BL, D)
        m["time_emb"] = time_e[sl].reshape(BL, D)
        m["noisy"] = noisy[sl].reshape(R, P * 2)
        m["offs"] = offs[sl].reshape(R, P * 2)
        m["w4"] = w4[sl].reshape(R, P * 4)
        in_maps.append(m)
    return in_maps


def kernel(**inputs):
    nc = _get_prog()
    in_maps = host_prep(inputs)
    res = bass_utils.run_bass_kernel_spmd(nc, in_maps, core_ids=list(range(NCORES)))
    outs = [np.asarray(r["out"]) for r in res.results]
    poses = np.concatenate([o[:, :24].reshape(BL, M, P, 3) for o in outs], axis=0)
    cls = np.concatenate([o[:, 24].reshape(BL, M) for o in outs], axis=0)
    return poses.astype(np.float32), cls.astype(np.float32)


# revision 15
# speedup vs baseline: 1.0863x; 1.0252x over previous
"""Trainium2 Bass kernel for nn_CustomTransformerDecoderLayer_3753801416834.

Strategy (pure data-parallel over batch, 4 batches per core on 8 cores):
  * grid_sample commutes with the 1x1 value conv (both linear) -> gather only
    the 2x2 bilinear corner patches from bev_feature and project afterwards.
    Host passes bev in NHWC layout so one gather descriptor is 512 contiguous
    floats (2 x-adjacent pixels x 256 channels) -> 1280 descriptors/core.
  * ego attention has Lk=1 so softmax == 1: collapses to a broadcast linear.
  * point-attention weights fold into the bilinear corner weights (one 32-way
    weighted reduce per query), and bias-terms fold into K=1 matmuls.
  * agent attention is batched across 4 batches x 8 heads as block-diagonal
    [80 x 128] score matrices with a multiplicative 0/1 mask.
Host-side work is limited to sharding, layout permutation and integer corner
index/weight prep; all tensor math runs on-device.
"""

import sys

import numpy as np

for _p in ("/opt/trn_rl_repo",):
    if _p not in sys.path:
        sys.path.insert(0, _p)

import concourse.bass as bass  # noqa: E402
import concourse.mybir as mybir  # noqa: E402
import concourse.tile as tile  # noqa: E402
from concourse import bacc, bass_utils  # noqa: E402
from concourse.masks import make_identity  # noqa: E402

F32 = mybir.dt.float32
I32 = mybir.dt.int32
AX = mybir.AxisListType.X
OP = mybir.AluOpType
AF = mybir.ActivationFunctionType

NCORES = 8
B, M, P, D, A, FF = 32, 20, 8, 256, 32, 1024
BL = B // NCORES          # local batches per core = 4
R = BL * M                # token rows per core = 80
AT = BL * A               # agent tokens per core = 128
NH, DH = 8, 32            # heads, head dim
HW = 128                  # bev spatial size
BEV_RANGE = 32.0
SCALE = 1.0 / float(np.sqrt(DH))

# ---- bias table layout (host & device share) -------------------------------
BIAS_SPEC = [
    ("aq_bq", 256), ("aq_bk", 256), ("aq_bv", 256), ("aq_bo", 256),
    ("eq_bv", 256), ("eq_bo", 256),
    ("ffn_b1", 1024), ("ffn_b2", 256),
    ("mod_b", 512),
    ("cls_b1", 256), ("cls_b2", 256), ("cls_b3", 1),
    ("reg_b1", 256), ("reg_b2", 256), ("reg_b3", 24),
    ("bev_attn_b", 8), ("bev_out_b", 256),
]
BOFF = {}
_o = 0
for _n, _s in BIAS_SPEC:
    BOFF[_n] = _o
    _o += _s
NB = _o

LN_SPEC = ["n1_g", "n1_b", "n2_g", "n2_b", "n3_g", "n3_b",
           "cls_ln1_g", "cls_ln1_b", "cls_ln2_g", "cls_ln2_b"]
LOFF = {n: i * 256 for i, n in enumerate(LN_SPEC)}
NL = 256 * len(LN_SPEC)

WEIGHTS_2D = [  # name -> (K, N); loaded as K/128 chunks of [128, N]
    ("aq_wq", 256, 256), ("aq_wk", 256, 256), ("aq_wv", 256, 256),
    ("aq_wo", 256, 256), ("eq_wv", 256, 256), ("eq_wo", 256, 256),
    ("ffn_w1", 256, 1024), ("ffn_w2", 1024, 256), ("mod_w", 256, 512),
    ("cls_w1", 256, 256), ("cls_w2", 256, 256), ("cls_w3", 256, 1),
    ("reg_w1", 256, 256), ("reg_w2", 256, 256), ("reg_w3", 256, 24),
    ("bev_value_w", 256, 256), ("bev_out_w", 256, 256), ("bev_attn_w", 256, 8),
]


def build_program(enable_asserts: bool = False):
    nc = bacc.Bacc(
        "TRN2",
        target_bir_lowering=False,
        debug=False,
        enable_asserts=enable_asserts,
        num_devices=NCORES,
    )

    def din(name, shape, dtype=F32):
        return nc.dram_tensor(name, list(shape), dtype, kind="ExternalInput")

    bev_h = din("bev", (BL * HW * HW, D))
    traj_h = din("traj", (R, D))
    agents_h = din("agents", (AT, D))
    ego_h = din("ego", (BL, D))
    time_h = din("time_emb", (BL, D))
    noisy_h = din("noisy", (R, P * 2))
    offs_h = din("offs", (R, P * 2), I32)
    w4_h = din("w4", (R, P * 4))
    amask_h = din("amask", (R, NH * AT))
    bsel_h = din("bsel", (BL, R))
    bcols_h = din("bcols", (128, 16))
    bigb_h = din("bigb", (1, NB))
    lnv_h = din("lnv", (1, NL))
    bvp_h = din("bv_pack", (128, 2))
    w_h = {n: din(n, (k, nn)) for n, k, nn in WEIGHTS_2D}
    out_h = nc.dram_tensor("out", [R, 25], F32, kind="ExternalOutput")

    with tile.TileContext(nc) as tc:
        with (
            tc.tile_pool(name="w", bufs=1) as wp,
            tc.tile_pool(name="act", bufs=1) as ap_,
            tc.tile_pool(name="tmp", bufs=4) as tp,
            tc.tile_pool(name="tt", bufs=1) as ttp,
            tc.tile_pool(name="ps", bufs=8, space="PSUM") as pp,
        ):
            # ---------------- constants / weights ----------------
            ident = wp.tile([128, 128], F32, tag="ident")
            make_identity(nc, ident[:])
            ones = wp.tile([1, 128], F32, tag="ones")
            nc.gpsimd.memset(ones[:], 1.0)
            epsc = wp.tile([128, 1], F32, tag="epsc")
            nc.gpsimd.memset(epsc[:], 1e-5)

            W = {}
            for wi, (name, K, N) in enumerate(WEIGHTS_2D):
                nchunk = K // 128
                t = wp.tile([128, nchunk * N], F32, tag=f"w_{name}", name=f"w_{name}")
                eng = nc.sync if wi % 2 == 0 else nc.scalar
                eng.dma_start(
                    out=t[:].rearrange("p (c n) -> p c n", c=nchunk),
                    in_=w_h[name].ap().rearrange("(c p) n -> p c n", p=128))
                W[name] = [t[:, i * N:(i + 1) * N] for i in range(nchunk)]

            bigb = wp.tile([1, NB], F32, tag="bigb")
            nc.sync.dma_start(out=bigb[:], in_=bigb_h.ap())
            bvp = wp.tile([128, 2], F32, tag="bvp")
            nc.sync.dma_start(out=bvp[:], in_=bvp_h.ap())

            def bslice(name, lo=0, size=None):
                off = BOFF[name] + lo
                if size is None:
                    size = dict((n, s) for n, s in BIAS_SPEC)[name]
                return bigb[0:1, off:off + size]

            # LN vectors replicated across partitions by a broadcast DMA
            lnr = wp.tile([128, NL], F32, tag="lnr")
            nc.scalar.dma_start(out=lnr[:], in_=lnv_h.ap().partition_broadcast(128))

            def lslice(name):
                return lnr[:R, LOFF[name]:LOFF[name] + 256]


            # ---------------- inputs ----------------
            _ldc = [0]
            def load(h, shape, dtype=F32, tag=None):
                t = ap_.tile(list(shape), dtype, tag=tag or h.name, name=tag or h.name)
                eng = nc.sync if _ldc[0] % 2 == 0 else nc.scalar
                _ldc[0] += 1
                eng.dma_start(out=t[:], in_=h.ap())
                return t

            traj = load(traj_h, (R, D))
            agents = load(agents_h, (AT, D))
            ego = load(ego_h, (BL, D))
            time_e = load(time_h, (BL, D))
            noisy = load(noisy_h, (R, P * 2))
            offs = load(offs_h, (R, P * 2), I32)
            w4 = load(w4_h, (R, P * 4))
            amask = load(amask_h, (R, NH * AT))
            bsel = load(bsel_h, (BL, R))
            bcols = load(bcols_h, (128, 16))

            # gathered 2x2xC corner patches: per (query,point,ycorner) 512 floats
            # HW indirect DMA consumes ONE index per partition, gathering the
            # out-row free size contiguously -> one call per (point, ycorner).
            G = ap_.tile([R, P * 2 * 512], F32, tag="G")
            for k in range(P * 2):
                nc.gpsimd.indirect_dma_start(
                    out=G[:, k * 512:(k + 1) * 512], out_offset=None,
                    in_=bev_h.ap(),
                    in_offset=bass.IndirectOffsetOnAxis(ap=offs[:, k:k + 1], axis=0),
                )

            # ---------------- helpers ----------------
            def PS(p, f):
                return pp.tile([p, f], F32, tag="ps", name="ps")

            def sb(p, f, tag):
                return ap_.tile([p, f], F32, tag=tag, name=tag)

            def copy(dst_ap, src_ap, eng="v"):
                if eng == "v":
                    nc.vector.tensor_copy(out=dst_ap, in_=src_ap)
                else:
                    nc.scalar.copy(out=dst_ap, in_=src_ap)

            def transpose_to(src_ap, pdim, fdim, tag, eng="v"):
                """src [pdim, fdim] (sbuf) -> new sbuf tile [fdim, pdim]."""
                ps = PS(fdim, pdim)
                nc.tensor.transpose(out=ps[:], in_=src_ap, identity=ident[:pdim, :pdim])
                t = ttp.tile([fdim, pdim], F32, tag=tag, name=tag)
                copy(t[:], ps[:], eng)
                return t

            def transpose256(src, pdim, tagbase, eng="v"):
                return [transpose_to(src[:, i * 128:(i + 1) * 128], pdim, 128,
                                     f"{tagbase}{i}", eng) for i in range(2)]

            def tok_bias(ps_ap, name, n, start=False, stop=False, rows=R):
                """out[m, :] += bias (K=1 matmul, ones as lhsT)."""
                nc.tensor.matmul(out=ps_ap, lhsT=ones[0:1, :rows],
                                 rhs=bslice(name, 0, n), start=start, stop=stop)

            def layer_norm(src_ap, gname, bname, dst_ap, sfx):
                srow = sb(R, 1, f"ln_s{sfx}")
                nc.vector.reduce_sum(srow[:], src_ap, axis=AX)
                nm = sb(R, 1, f"ln_nm{sfx}")
                nc.scalar.activation(out=nm[:], in_=srow[:], func=AF.Copy,
                                     scale=-1.0 / D)
                xm = sb(R, D, f"ln_xm{sfx}")
                nc.vector.tensor_scalar_add(out=xm[:], in0=src_ap, scalar1=nm[:])
                sq = tp.tile([R, D], F32, tag="fmatmp", name="fmatmp")
                ssq = sb(R, 1, f"ln_q{sfx}")
                nc.scalar.activation(out=sq[:], in_=xm[:], func=AF.Square,
                                     accum_out=ssq[:])
                # rsqrt(var+eps) on DVE only: quake seed + 2 Newton steps
                v_ = sb(R, 1, f"ln_v{sfx}")
                nc.vector.tensor_scalar(out=v_[:], in0=ssq[:], scalar1=1.0 / D,
                                        scalar2=1e-5, op0=OP.mult, op1=OP.add)
                iv = sb(R, 1, f"ln_iv{sfx}")
                I32v = v_[:].bitcast(I32)
                nc.vector.tensor_scalar(out=iv[:].bitcast(I32), in0=I32v,
                                        scalar1=1, scalar2=None,
                                        op0=OP.arith_shift_right)
                rstd = sb(R, 1, f"ln_r{sfx}")
                nc.vector.tensor_scalar(out=rstd[:].bitcast(I32),
                                        in0=iv[:].bitcast(I32),
                                        scalar1=-1, scalar2=0x5f3759df,
                                        op0=OP.mult, op1=OP.add)
                for it_ in range(2):
                    yy = sb(R, 1, f"ln_y{sfx}{it_}")
                    nc.vector.tensor_tensor(out=yy[:], in0=rstd[:], in1=rstd[:],
                                            op=OP.mult)
                    nc.vector.tensor_tensor(out=yy[:], in0=yy[:], in1=v_[:],
                                            op=OP.mult)
                    nc.vector.tensor_scalar(out=yy[:], in0=yy[:], scalar1=-0.5,
                                            scalar2=1.5, op0=OP.mult, op1=OP.add)
                    nc.vector.tensor_tensor(out=rstd[:], in0=rstd[:], in1=yy[:],
                                            op=OP.mult)
                nc.vector.tensor_scalar_mul(out=xm[:], in0=xm[:], scalar1=rstd[:])
                nc.vector.tensor_tensor(out=dst_ap, in0=xm[:], in1=lslice(gname),
                                        op=OP.mult)
                nc.vector.tensor_tensor(out=dst_ap, in0=dst_ap, in1=lslice(bname),
                                        op=OP.add)

            # ---------------- BEV branch ----------------
            # point-attention weights ptw = softmax(traj @ bev_attn_w + b)
            trajT = transpose256(traj, R, "trajT")
            pw_ps = PS(R, P)
            nc.tensor.matmul(out=pw_ps[:], lhsT=trajT[0][:, :R], rhs=W["bev_attn_w"][0],
                             start=True, stop=False)
            nc.tensor.matmul(out=pw_ps[:], lhsT=trajT[1][:, :R], rhs=W["bev_attn_w"][1],
                             start=False, stop=False)
            tok_bias(pw_ps[:], "bev_attn_b", P, stop=True)
            nmax = sb(R, 1, "pw_nmax")
            nc.vector.reduce_max(nmax[:], pw_ps[:], axis=AX, negate=True)
            pexp = sb(R, P, "pw_exp")
            nc.scalar.activation(out=pexp[:], in_=pw_ps[:], func=AF.Exp,
                                 bias=nmax[:], scale=1.0)
            psum_ = sb(R, 1, "pw_sum")
            nc.vector.reduce_sum(psum_[:], pexp[:], axis=AX)
            prec = sb(R, 1, "pw_rec")
            nc.vector.reciprocal(prec[:], psum_[:])
            ptw = sb(R, P, "ptw")
            nc.vector.tensor_scalar_mul(out=ptw[:], in0=pexp[:], scalar1=prec[:])

            # combined corner weights cw[m, pt*4 + yc*2 + xc]
            cw = sb(R, P * 4, "cw")
            for p_ in range(P):
                nc.vector.tensor_scalar_mul(out=cw[:, 4 * p_:4 * p_ + 4],
                                            in0=w4[:, 4 * p_:4 * p_ + 4],
                                            scalar1=ptw[:, p_:p_ + 1])

            # gq[m, c] = sum_j cw[m, j] * G[m, j, c]   (4 parallel chains)
            accs = []
            for k in range(4):
                a = sb(R, D, f"gacc{k}")
                for ji, j in enumerate(range(k * 8, k * 8 + 8)):
                    pt, rem = divmod(j, 4)
                    yc, xc = divmod(rem, 2)
                    off = (pt * 2 + yc) * 512 + xc * 256
                    gsl = G[:, off:off + 256]
                    if ji == 0:
                        nc.scalar.activation(out=a[:], in_=gsl, func=AF.Copy,
                                             scale=cw[:, j:j + 1])
                    else:
                        t = tp.tile([R, D], F32, tag="fmatmp", name="fmatmp")
                        nc.scalar.activation(out=t[:], in_=gsl, func=AF.Copy,
                                             scale=cw[:, j:j + 1])
                        nc.vector.tensor_tensor(out=a[:], in0=a[:], in1=t[:], op=OP.add)
                accs.append(a)
            nc.vector.tensor_tensor(out=accs[0][:], in0=accs[0][:], in1=accs[1][:], op=OP.add)
            nc.vector.tensor_tensor(out=accs[2][:], in0=accs[2][:], in1=accs[3][:], op=OP.add)
            gq = sb(R, D, "gq")
            nc.vector.tensor_tensor(out=gq[:], in0=accs[0][:], in1=accs[2][:], op=OP.add)

            # s[m] = sum_j cw[m, j]  (validity-weighted bias scale), as [1, R]
            srow = sb(R, 1, "cw_s")
            nc.vector.reduce_sum(srow[:], cw[:], axis=AX)
            sT_ps = PS(1, R)
            nc.tensor.transpose(out=sT_ps[:], in_=srow[:, 0:1], identity=ident[:R, :R])
            sT = sb(1, R, "cw_sT")
            copy(sT[:], sT_ps[:], "s")

            # W2 = bev_value_w @ bev_out_w  (on device, one-time)
            WvT = []  # e-chunks [128e, 256c]
            for j in range(2):
                t = wp.tile([128, 256], F32, tag=f"WvT{j}", name=f"WvT{j}")
                for i in range(2):
                    ps = PS(128, 128)
                    nc.tensor.transpose(out=ps[:],
                                        in_=W["bev_value_w"][i][:, j * 128:(j + 1) * 128],
                                        identity=ident[:])
                    copy(t[:, i * 128:(i + 1) * 128], ps[:], "s")
                WvT.append(t)
            W2 = []
            for i in range(2):
                ps = PS(128, 256)
                for j in range(2):
                    nc.tensor.matmul(out=ps[:], lhsT=WvT[j][:, i * 128:(i + 1) * 128],
                                     rhs=W["bev_out_w"][j],
                                     start=(j == 0), stop=(j == 1))
                t = wp.tile([128, 256], F32, tag=f"W2_{i}", name=f"W2_{i}")
                copy(t[:], ps[:], "v")
                W2.append(t)

            # bvWo = bev_value_b @ bev_out_w  [1, 256]
            bw_ps = PS(1, 256)
            nc.tensor.matmul(out=bw_ps[:], lhsT=bvp[:, 0:1], rhs=W["bev_out_w"][0],
                             start=True, stop=False)
            nc.tensor.matmul(out=bw_ps[:], lhsT=bvp[:, 1:2], rhs=W["bev_out_w"][1],
                             start=False, stop=True)
            bvWo = sb(1, 256, "bvWo")
            copy(bvWo[:], bw_ps[:], "s")

            # traj1 = traj + gq @ W2 + s * bvWo + bev_out_b
            gqT = transpose256(gq, R, "gqT")
            t1_ps = PS(R, D)
            nc.tensor.matmul(out=t1_ps[:], lhsT=gqT[0][:, :R], rhs=W2[0][:], start=True, stop=False)
            nc.tensor.matmul(out=t1_ps[:], lhsT=gqT[1][:, :R], rhs=W2[1][:], start=False, stop=False)
            nc.tensor.matmul(out=t1_ps[:], lhsT=sT[0:1, :R], rhs=bvWo[0:1, :], start=False, stop=False)
            tok_bias(t1_ps[:], "bev_out_b", 256)
            nc.tensor.matmul(out=t1_ps[:], lhsT=ident[:R, :R], rhs=traj[:], start=False, stop=True)
            traj1 = sb(R, D, "traj1")
            copy(traj1[:], t1_ps[:], "v")

            # ---------------- agents cross-attention + LN1 ----------------
            agT = transpose256(agents, AT, "agT")
            tr1T = transpose256(traj1, R, "tr1T")

            def projT(wname, bname, rhs_tiles, ntok, tagbase):
                """x @ W + b, produced transposed: 4 chunks [64 dout, ntok]
                so per-head slices start at partition base 0 or 32."""
                outs = []
                bc0 = 0 if wname == "aq_wq" else 4
                for j in range(4):
                    ps = PS(64, ntok)
                    for i in range(2):
                        nc.tensor.matmul(out=ps[:],
                                         lhsT=W[wname][i][:, j * 64:(j + 1) * 64],
                                         rhs=rhs_tiles[i][:, :ntok],
                                         start=(i == 0), stop=(i == 1))
                    t = ttp.tile([64, ntok], F32, tag=f"{tagbase}{j}", name=f"{tagbase}{j}")
                    nc.scalar.activation(out=t[:], in_=ps[:], func=AF.Identity,
                                         bias=bcols[:64, bc0 + j:bc0 + j + 1])
                    outs.append(t)
                return outs

            qT = projT("aq_wq", "aq_bq", tr1T, R, "qT")
            kT = projT("aq_wk", "aq_bk", agT, AT, "kT")

            # v_proj [AT, 256]
            v_ps = PS(AT, 256)
            nc.tensor.matmul(out=v_ps[:], lhsT=agT[0][:], rhs=W["aq_wv"][0], start=True, stop=False)
            nc.tensor.matmul(out=v_ps[:], lhsT=agT[1][:], rhs=W["aq_wv"][1], start=False, stop=False)
            tok_bias(v_ps[:], "aq_bv", 256, stop=True, rows=AT)
            vproj = sb(AT, 256, "vproj")
            copy(vproj[:], v_ps[:], "v")

            # scores -> masked softmax (no max-sub needed; values bounded)
            eall = sb(R, NH * AT, "eall")
            for h in range(NH):
                j, r0 = divmod(h, 2)
                r0 *= DH
                s_ps = PS(R, AT)
                nc.tensor.matmul(out=s_ps[:], lhsT=qT[j][r0:r0 + DH, :R],
                                 rhs=kT[j][r0:r0 + DH, :AT], start=True, stop=True)
                nc.scalar.activation(out=eall[:, h * AT:(h + 1) * AT], in_=s_ps[:],
                                     func=AF.Exp, scale=SCALE)
            nc.vector.tensor_tensor(out=eall[:], in0=eall[:], in1=amask[:], op=OP.mult)
            rs = sb(R, NH, "rs")
            nc.vector.reduce_sum(rs[:], eall[:].rearrange("p (h k) -> p h k", k=AT), axis=AX)
            rin = sb(R, NH, "rin")
            nc.vector.reciprocal(rin[:], rs[:])
            pn = sb(R, NH * AT, "pn")
            for h in range(NH):
                nc.vector.tensor_scalar_mul(out=pn[:, h * AT:(h + 1) * AT],
                                            in0=eall[:, h * AT:(h + 1) * AT],
                                            scalar1=rin[:, h:h + 1])

            attnT_sb = []
            for j in range(2):
                t = ttp.tile([128, R], F32, tag=f"attnT{j}", name=f"attnT{j}")
                for hh in range(4):
                    h = j * 4 + hh
                    pT_ps = PS(AT, R)
                    nc.tensor.transpose(out=pT_ps[:], in_=pn[:, h * AT:(h + 1) * AT],
                                        identity=ident[:R, :R])
                    pT = tp.tile([AT, R], F32, tag="pT", name="pT")
                    copy(pT[:], pT_ps[:], "s" if hh % 2 else "v")
                    a_ps = PS(DH, R)
                    nc.tensor.matmul(out=a_ps[:],
                                     lhsT=vproj[:, h * DH:(h + 1) * DH],
                                     rhs=pT[:], start=True, stop=True)
                    copy(t[hh * DH:(hh + 1) * DH, :], a_ps[:],
                         "v" if hh % 2 else "s")
                attnT_sb.append(t)

            o_ps = PS(R, D)
            nc.tensor.matmul(out=o_ps[:], lhsT=attnT_sb[0][:], rhs=W["aq_wo"][0], start=True, stop=False)
            nc.tensor.matmul(out=o_ps[:], lhsT=attnT_sb[1][:], rhs=W["aq_wo"][1], start=False, stop=False)
            tok_bias(o_ps[:], "aq_bo", 256)
            nc.tensor.matmul(out=o_ps[:], lhsT=ident[:R, :R], rhs=traj1[:], start=False, stop=True)
            traj2 = sb(R, D, "traj2")
            layer_norm(o_ps[:], "n1_g", "n1_b", traj2[:], "1")

            # ---------------- ego branch (softmax over 1 key == 1) + LN2 ----
            egoT = transpose256(ego, BL, "egoT")
            ev_ps = PS(BL, 256)
            nc.tensor.matmul(out=ev_ps[:], lhsT=egoT[0][:, :BL], rhs=W["eq_wv"][0], start=True, stop=False)
            nc.tensor.matmul(out=ev_ps[:], lhsT=egoT[1][:, :BL], rhs=W["eq_wv"][1], start=False, stop=False)
            tok_bias(ev_ps[:], "eq_bv", 256, stop=True, rows=BL)
            ev = sb(BL, 256, "ev")
            copy(ev[:], ev_ps[:], "s")
            evT = transpose256(ev, BL, "evT")
            e_ps = PS(BL, 256)
            nc.tensor.matmul(out=e_ps[:], lhsT=evT[0][:, :BL], rhs=W["eq_wo"][0], start=True, stop=False)
            nc.tensor.matmul(out=e_ps[:], lhsT=evT[1][:, :BL], rhs=W["eq_wo"][1], start=False, stop=False)
            tok_bias(e_ps[:], "eq_bo", 256, stop=True, rows=BL)
            eout = sb(BL, 256, "eout")
            copy(eout[:], e_ps[:], "s")
            ebc_ps = PS(R, 256)
            nc.tensor.matmul(out=ebc_ps[:], lhsT=bsel[:, :R], rhs=eout[:],
                             start=True, stop=True)
            t3p = sb(R, D, "t3p")
            nc.vector.tensor_tensor(out=t3p[:], in0=traj2[:], in1=ebc_ps[:], op=OP.add)
            traj3 = sb(R, D, "traj3")
            layer_norm(t3p[:], "n2_g", "n2_b", traj3[:], "2")

            # ---------------- FFN + LN3 (no residual) ----------------
            tr3T = transpose256(traj3, R, "tr3T")
            hT = []
            for f in range(FF // 128):
                h_ps = PS(128, R)
                for i in range(2):
                    nc.tensor.matmul(out=h_ps[:],
                                     lhsT=W["ffn_w1"][i][:, f * 128:(f + 1) * 128],
                                     rhs=tr3T[i][:, :R], start=(i == 0), stop=(i == 1))
                t = ttp.tile([128, R], F32, tag=f"hT{f % 4}_{f // 4}", name=f"hT{f}")
                nc.scalar.activation(out=t[:], in_=h_ps[:], func=AF.Relu,
                                     bias=bcols[:, 8 + f:9 + f])
                hT.append(t)
            f2_ps = PS(R, D)
            for f in range(FF // 128):
                nc.tensor.matmul(out=f2_ps[:], lhsT=hT[f][:], rhs=W["ffn_w2"][f],
                                 start=(f == 0), stop=False)
            tok_bias(f2_ps[:], "ffn_b2", 256, stop=True)
            traj4a = sb(R, D, "traj4a")
            layer_norm(f2_ps[:], "n3_g", "n3_b", traj4a[:], "3")

            # ---------------- FiLM time modulation ----------------
            mish = sb(BL, D, "mish")
            mt = sb(BL, D, "mish_t")
            nc.scalar.activation(out=mt[:], in_=time_e[:], func=AF.Exp)
            mu = sb(BL, D, "mish_u")
            nc.scalar.activation(out=mu[:], in_=mt[:], func=AF.Square, bias=1.0)
            mden = sb(BL, D, "mish_d")
            nc.vector.tensor_scalar_add(out=mden[:], in0=mu[:], scalar1=1.0)
            nc.vector.reciprocal(mden[:], mden[:])
            mnum = sb(BL, D, "mish_n")
            nc.vector.tensor_scalar_add(out=mnum[:], in0=mu[:], scalar1=-1.0)
            nc.vector.tensor_tensor(out=mnum[:], in0=mnum[:], in1=mden[:], op=OP.mult)
            nc.vector.tensor_tensor(out=mish[:], in0=time_e[:], in1=mnum[:], op=OP.mult)
            mishT = transpose256(mish, BL, "mishT")
            ss_ps = PS(BL, 512)
            nc.tensor.matmul(out=ss_ps[:], lhsT=mishT[0][:, :BL], rhs=W["mod_w"][0], start=True, stop=False)
            nc.tensor.matmul(out=ss_ps[:], lhsT=mishT[1][:, :BL], rhs=W["mod_w"][1], start=False, stop=False)
            tok_bias(ss_ps[:], "mod_b", 512, stop=True, rows=BL)
            ss = sb(BL, 512, "ss")
            copy(ss[:], ss_ps[:], "s")
            ssb_ps = PS(R, 512)
            nc.tensor.matmul(out=ssb_ps[:], lhsT=bsel[:, :R], rhs=ss[:],
                             start=True, stop=True)
            tf1 = sb(R, D, "tf1")
            nc.vector.tensor_tensor(out=tf1[:], in0=traj4a[:], in1=ssb_ps[:, :256], op=OP.mult)
            nc.vector.tensor_tensor(out=tf1[:], in0=tf1[:], in1=ssb_ps[:, 256:], op=OP.add)
            traj4 = sb(R, D, "traj4")
            nc.vector.tensor_tensor(out=traj4[:], in0=tf1[:], in1=traj4a[:], op=OP.add)

            # ---------------- heads ----------------
            out_sb = sb(R, 25, "out_sb")
            tr4T = transpose256(traj4, R, "tr4T")

            def linear_relu(in_T, wname, bname, tagbase, relu=True):
                ps = PS(R, D)
                for i in range(2):
                    nc.tensor.matmul(out=ps[:], lhsT=in_T[i][:, :R], rhs=W[wname][i],
                                     start=(i == 0), stop=False)
                tok_bias(ps[:], bname, 256, stop=True)
                t = sb(R, D, tagbase)
                if relu:
                    nc.scalar.activation(out=t[:], in_=ps[:], func=AF.Relu)
                else:
                    copy(t[:], ps[:], "v")
                return t

            # cls branch
            c1r = linear_relu(tr4T, "cls_w1", "cls_b1", "c1r")
            c1 = sb(R, D, "c1")
            layer_norm(c1r[:], "cls_ln1_g", "cls_ln1_b", c1[:], "c1")
            c1T = transpose256(c1, R, "c1T")
            c2r = linear_relu(c1T, "cls_w2", "cls_b2", "c2r")
            c2 = sb(R, D, "c2")
            layer_norm(c2r[:], "cls_ln2_g", "cls_ln2_b", c2[:], "c2")
            c2T = transpose256(c2, R, "c2T")
            cls_ps = PS(R, 1)
            nc.tensor.matmul(out=cls_ps[:], lhsT=c2T[0][:, :R], rhs=W["cls_w3"][0], start=True, stop=False)
            nc.tensor.matmul(out=cls_ps[:], lhsT=c2T[1][:, :R], rhs=W["cls_w3"][1], start=False, stop=False)
            tok_bias(cls_ps[:], "cls_b3", 1, stop=True)
            nc.scalar.copy(out=out_sb[:, 24:25], in_=cls_ps[:])

            # reg branch
            r1 = linear_relu(tr4T, "reg_w1", "reg_b1", "r1")
            r1T = transpose256(r1, R, "r1T")
            r2 = linear_relu(r1T, "reg_w2", "reg_b2", "r2")
            r2T = transpose256(r2, R, "r2T")
            r3_ps = PS(R, 24)
            nc.tensor.matmul(out=r3_ps[:], lhsT=r2T[0][:, :R], rhs=W["reg_w3"][0], start=True, stop=False)
            nc.tensor.matmul(out=r3_ps[:], lhsT=r2T[1][:, :R], rhs=W["reg_w3"][1], start=False, stop=False)
            tok_bias(r3_ps[:], "reg_b3", 24, stop=True)

            r3v = r3_ps[:].rearrange("p (t c) -> p t c", c=3)
            outv = out_sb[:, 0:24].rearrange("p (t c) -> p t c", c=3)
            noisyv = noisy[:].rearrange("p (t c) -> p t c", c=2)
            nc.vector.tensor_tensor(out=outv[:, :, 0:2], in0=r3v[:, :, 0:2],
                                    in1=noisyv, op=OP.add)
            th = sb(R, P, "th")
            nc.scalar.activation(out=th[:], in_=r3v[:, :, 2], func=AF.Tanh)
            nc.vector.tensor_scalar_mul(out=outv[:, :, 2], in0=th[:],
                                        scalar1=float(np.pi))

            nc.sync.dma_start(out=out_h.ap(), in_=out_sb[:])

    nc.compile()
    return nc


# ---------------------------------------------------------------------------
# host side
# ---------------------------------------------------------------------------
_PROG = None


def _get_prog():
    global _PROG
    if _PROG is None:
        _PROG = build_program()
    return _PROG


def host_prep(inputs):
    f32, i32 = np.float32, np.int32
    traj = np.ascontiguousarray(np.asarray(inputs["traj_feature"], f32))
    noisy = np.ascontiguousarray(np.asarray(inputs["noisy_traj_points"], f32))
    bev = np.asarray(inputs["bev_feature"], f32)
    agents = np.ascontiguousarray(np.asarray(inputs["agents_query"], f32))
    ego = np.ascontiguousarray(np.asarray(inputs["ego_query"], f32))
    time_e = np.ascontiguousarray(np.asarray(inputs["time_emb"] if "time_emb" in inputs
                                             else inputs["time_embed"], f32))

    # bilinear corner indices / weights (host: pure index arithmetic)
    g = noisy / BEV_RANGE                      # (B, M, P, 2)
    x = (g[..., 0] + 1.0) * (HW * 0.5) - 0.5
    y = (g[..., 1] + 1.0) * (HW * 0.5) - 0.5
    x0 = np.floor(x).astype(np.int64)
    y0 = np.floor(y).astype(np.int64)
    wx = np.stack([x0 + 1 - x, x - x0], -1)    # (B,M,P,2) weights for x0, x0+1
    wy = np.stack([y0 + 1 - y, y - y0], -1)
    xg = np.clip(x0, 0, HW - 2)
    # slot weights for the gathered x-pair [xg, xg+1]
    wxs = np.zeros(x.shape + (2,), f32)
    for k in (0, 1):
        xi = x0 + k
        valid = (xi >= 0) & (xi <= HW - 1)
        for s in (0, 1):
            wxs[..., s] += np.where(valid & (xi == xg + s), wx[..., k], 0.0)
    # y rows gathered: clip(y0,0,127) and clip(y0+1,0,127), zero-weight if OOB
    ys = np.stack([np.clip(y0, 0, HW - 1), np.clip(y0 + 1, 0, HW - 1)], -1)
    wys = np.stack([np.where((y0 >= 0) & (y0 <= HW - 1), wy[..., 0], 0.0),
                    np.where((y0 + 1 >= 0) & (y0 + 1 <= HW - 1), wy[..., 1], 0.0)], -1)
    # w4[b,m,pt,yc,xc], offs[b,m,pt,yc]
    w4 = (wys[..., :, None] * wxs[..., None, :]).astype(f32)      # (B,M,P,2,2)
    # element offset into per-core NHWC shard: ((b_loc*HW + y)*HW + xg)*D
    b_loc = (np.arange(B) % BL)[:, None, None, None]
    offs = (b_loc * HW + ys) * HW + xg[..., None]                 # pixel index
    offs = offs.astype(i32)

    # agent-attention block mask (same for every core)
    amask = np.zeros((R, NH * AT), f32)
    for p_ in range(R):
        bi = p_ // M
        for h in range(NH):
            amask[p_, h * AT + bi * A: h * AT + bi * A + A] = 1.0

    # bias table / LN table / weights (shared across cores)
    bigb = np.zeros((1, NB), f32)
    for n, s_ in BIAS_SPEC:
        key = {"bev_attn_b": "bev_attn_b", "bev_out_b": "bev_out_b"}.get(n, n)
        bigb[0, BOFF[n]:BOFF[n] + s_] = np.asarray(inputs[key], f32).reshape(-1)
    lnv = np.concatenate([np.asarray(inputs[n], f32).reshape(-1) for n in LN_SPEC])[None]
    bvp = np.asarray(inputs["bev_value_b"], f32).reshape(2, 128).T.copy()  # [128, 2]

    bcols = np.zeros((128, 16), f32)
    for j in range(4):
        bcols[:64, j] = np.asarray(inputs["aq_bq"], f32)[j * 64:(j + 1) * 64]
        bcols[:64, 4 + j] = np.asarray(inputs["aq_bk"], f32)[j * 64:(j + 1) * 64]
    for f in range(8):
        bcols[:, 8 + f] = np.asarray(inputs["ffn_b1"], f32)[f * 128:(f + 1) * 128]

    bsel = np.zeros((BL, R), f32)
    for b_ in range(BL):
        bsel[b_, b_ * M:(b_ + 1) * M] = 1.0

    shared = {"bigb": bigb, "lnv": np.ascontiguousarray(lnv), "bv_pack": bvp,
              "amask": amask, "bsel": bsel, "bcols": bcols}
    for n, _k, _n2 in WEIGHTS_2D:
        shared[n] = np.ascontiguousarray(np.asarray(inputs[n], f32))

    in_maps = []
    for c in range(NCORES):
        sl = slice(c * BL, (c + 1) * BL)
        m = dict(shared)
        m["bev"] = np.ascontiguousarray(bev[sl].transpose(0, 2, 3, 1)).reshape(-1, D)
        m["traj"] = traj[sl].reshape(R, D)
        m["agents"] = agents[sl].reshape(AT, D)
        m["ego"] = ego[sl].reshape(BL, D)
        m["time_emb"] = time_e[sl].reshape(BL, D)
        m["noisy"] = noisy[sl].reshape(R, P * 2)
        m["offs"] = offs[sl].reshape(R, P * 2)
        m["w4"] = w4[sl].reshape(R, P * 4)
        in_maps.append(m)
    return in_maps


def kernel(**inputs):
    nc = _get_prog()
    in_maps = host_prep(inputs)
    res = bass_utils.run_bass_kernel_spmd(nc, in_maps, core_ids=list(range(NCORES)))
    outs = [np.asarray(r["out"]) for r in res.results]
    poses = np.concatenate([o[:, :24].reshape(BL, M, P, 3) for o in outs], axis=0)
    cls = np.concatenate([o[:, 24].reshape(BL, M) for o in outs], axis=0)
    return poses.astype(np.float32), cls.astype(np.float32)
